# revision 11
# baseline (speedup 1.0000x reference)
"""HSTU-style dense transformer for sequence modeling on 8 Trainium2 NeuronCores.

Sharding: data-parallel over batch (B=8 -> 1 sequence per core). All weights
replicated. Activations are kept feature-major [D=partitions, T=free] on chip so
every GEMM consumes weights as stationary lhsT tiles and activations as the
moving operand; attention computes transposed scores [kt, qt] so the
sigmoid/causal-trim/AV chain needs no on-device transposes.

All matmul operands are bf16 (weights pre-cast on host): bf16 gets the fast
weight-load path so LDWEIGHTS hides under the previous matmul's streaming, and
weight DMA traffic halves vs f32. The residual stream x and all PSUM
accumulation stay fp32; a bf16 shadow copy of x feeds the LayerNorm-stats
matmuls and the final logits GEMM. The two 64-wide heads of each q/k feature
chunk run concurrently on the PE via row tiling (scores, disjoint row groups)
and col tiling (AV, explicit tile_position), writing one [128,S] PSUM tile.

Host side only marshals: embedding gather + positional add, weight pre-tiling
into DMA-contiguous bf16 layouts, and the final [V,T] -> [S,V] untranspose.
"""

import sys

sys.path.insert(0, "/opt/trn_rl_repo")

import numpy as np

import concourse.bass as bass  # noqa: F401  (keeps bass registered before bacc)
import concourse.tile as tile
from concourse import bacc, mybir
from concourse.bass import ts
from concourse.bass_utils import run_bass_kernel_spmd

B, S, D, H, L, V = 8, 512, 1024, 16, 6, 32000
DH = D // H
LN_EPS = 1e-5
N_CORES = 8
NC_D = D // 128      # 8 feature chunks
NC_T = S // 128      # 4 token chunks
NC_V = V // 128      # 250 vocab chunks
NP = 8               # head pairs

F32 = mybir.dt.float32
BF16 = mybir.dt.bfloat16
BF16_NP = mybir.dt.np(mybir.dt.bfloat16)
AF = mybir.ActivationFunctionType
OP = mybir.AluOpType

_prog_cache = {}


def _build(cfg):
    """Build + compile the SPMD per-core program. cfg is a hashable dict-key."""
    (use_lng, use_lnb, use_bqk, use_bv, use_bg, use_bo, use_bp, rpb_nz) = cfg
    DT = BF16

    nc = bacc.Bacc("TRN2", target_bir_lowering=False, debug=False)

    x0_d = nc.dram_tensor("x0t", [NC_D, 128, S], F32, kind="ExternalInput").ap()
    wqk_d = nc.dram_tensor("wqk", [L, 16, 128, 1024], DT, kind="ExternalInput").ap()
    wv_d = nc.dram_tensor("wv", [L, 16, 128, 512], DT, kind="ExternalInput").ap()
    wg_d = nc.dram_tensor("wg", [L, NC_D, 128, 1024], DT, kind="ExternalInput").ap()
    wo_d = nc.dram_tensor("wo", [L, NC_D, 128, 1024], DT, kind="ExternalInput").ap()
    wp_d = nc.dram_tensor("wp", [NC_V, 128, 1024], DT, kind="ExternalInput").ap()
    tri_d = nc.dram_tensor("tri", [128, 128], DT, kind="ExternalInput").ap()
    onec_d = nc.dram_tensor("onec", [128, 1], DT, kind="ExternalInput").ap()
    oner_d = nc.dram_tensor("oner", [1, 128], DT, kind="ExternalInput").ap()
    lng_d = nc.dram_tensor("lng", [L, NC_D, 128], F32, kind="ExternalInput").ap() if use_lng else None
    lnb_d = nc.dram_tensor("lnb", [L, NC_D, 128], F32, kind="ExternalInput").ap() if use_lnb else None
    bqk_d = nc.dram_tensor("bqk", [L, 16, 128], F32, kind="ExternalInput").ap() if use_bqk else None
    bv_d = nc.dram_tensor("bv", [L, 2, 1, 512], DT, kind="ExternalInput").ap() if use_bv else None
    bg_d = nc.dram_tensor("bg", [L, NC_D, 128], F32, kind="ExternalInput").ap() if use_bg else None
    bo_d = nc.dram_tensor("bo", [L, NC_D, 128], F32, kind="ExternalInput").ap() if use_bo else None
    bp_d = nc.dram_tensor("bp", [NC_V, 128], F32, kind="ExternalInput").ap() if use_bp else None
    rpb_d = nc.dram_tensor("rpb", [1, H * L], DT, kind="ExternalInput").ap() if rpb_nz else None
    out_d = nc.dram_tensor("logits_t", [NC_V, 128, S], F32, kind="ExternalOutput").ap()

    with tile.TileContext(nc) as tc, nc.allow_low_precision(
        reason="bf16 tiles feed the PE; accumulation stays fp32 in PSUM"
    ):
        from contextlib import ExitStack

        with ExitStack() as ctx:
            cp = ctx.enter_context(tc.tile_pool(name="consts", bufs=1))
            xp = ctx.enter_context(tc.tile_pool(name="x", bufs=2))
            xbp = ctx.enter_context(tc.tile_pool(name="xb", bufs=1))
            xnp = ctx.enter_context(tc.tile_pool(name="xn", bufs=1))
            up = ctx.enter_context(tc.tile_pool(name="u", bufs=1))
            vp = ctx.enter_context(tc.tile_pool(name="v", bufs=1))
            gp = ctx.enter_context(tc.tile_pool(name="g", bufs=1))
            qkp = ctx.enter_context(tc.tile_pool(name="qk", bufs=2))
            tmp = ctx.enter_context(tc.tile_pool(name="tmp", bufs=3))
            rows = ctx.enter_context(tc.tile_pool(name="rows", bufs=4))
            bp_pool = ctx.enter_context(tc.tile_pool(name="bcast", bufs=4))
            atp = ctx.enter_context(tc.tile_pool(name="at", bufs=4))
            dtp = ctx.enter_context(tc.tile_pool(name="dg", bufs=2))
            wbp = ctx.enter_context(tc.tile_pool(name="wb", bufs=4))
            wvp = ctx.enter_context(tc.tile_pool(name="wvp", bufs=1))
            op_pool = ctx.enter_context(tc.tile_pool(name="out", bufs=4))
            prm = ctx.enter_context(tc.tile_pool(name="prm", bufs=2))
            pmm = ctx.enter_context(tc.tile_pool(name="pmm", bufs=3, space="PSUM"))
            pao = ctx.enter_context(tc.tile_pool(name="pao", bufs=1, space="PSUM"))
            psc = ctx.enter_context(tc.tile_pool(name="psc", bufs=2, space="PSUM"))
            pst = ctx.enter_context(tc.tile_pool(name="pst", bufs=2, space="PSUM"))

            mm = nc.tensor.matmul

            onec = cp.tile([128, 1], DT)
            nc.sync.dma_start(onec[:], onec_d[:])
            oner = cp.tile([1, 128], DT)
            nc.sync.dma_start(oner[:], oner_d[:])
            tri_t = cp.tile([128, 128], DT)
            nc.sync.dma_start(tri_t[:], tri_d[:])
            if rpb_nz:
                rpb_row = cp.tile([1, H * L], DT)
                nc.sync.dma_start(rpb_row[:], rpb_d[:])
                # broadcast to [128, H*L] so column slices give per-partition bias
                prb = psc.tile([128, 512], F32, tag="sc")
                mm(prb[:, : H * L], oner[:], rpb_row[:], start=True, stop=True)
                rpb_t = cp.tile([128, H * L], F32)
                nc.scalar.copy(rpb_t[:], prb[:, : H * L])
            if use_bp:
                bp_t = cp.tile([128, NC_V], F32)
                nc.sync.dma_start(bp_t[:], bp_d.rearrange("v p -> p v"))

            x_cur = xp.tile([128, NC_D * S], F32, tag="x")
            for c in range(NC_D):
                nc.sync.dma_start(x_cur[:, ts(c, S)], x0_d[c])
            # bf16 shadow of x: feeds LN-stats matmuls (and logits GEMM at the end)
            xb = xbp.tile([128, NC_D * S], DT, tag="xb")
            ps_s = pst.tile([1, S], F32, tag="st")
            ps_q = pst.tile([1, S], F32, tag="st")
            for c in range(NC_D):
                nc.scalar.copy(xb[:, ts(c, S)], x_cur[:, ts(c, S)])
                sq = tmp.tile([128, S], DT, tag="sq")
                nc.vector.tensor_mul(sq[:], xb[:, ts(c, S)], xb[:, ts(c, S)])
                mm(ps_s[:], onec[:], xb[:, ts(c, S)], start=(c == 0), stop=(c == NC_D - 1))
                mm(ps_q[:], onec[:], sq[:], start=(c == 0), stop=(c == NC_D - 1))

            for l in range(L):
                # ---- per-layer params ----
                if use_lng:
                    lng_t = prm.tile([128, NC_D], F32, tag="lng")
                    nc.sync.dma_start(lng_t[:], lng_d[l].rearrange("c p -> p c"))
                if use_lnb:
                    lnb_t = prm.tile([128, NC_D], F32, tag="lnb")
                    nc.sync.dma_start(lnb_t[:], lnb_d[l].rearrange("c p -> p c"))
                if use_bqk:
                    bqk_t = prm.tile([128, 16], F32, tag="bqk")
                    nc.sync.dma_start(bqk_t[:], bqk_d[l].rearrange("c p -> p c"))
                if use_bg:
                    bg_t = prm.tile([128, NC_D], F32, tag="bg")
                    nc.sync.dma_start(bg_t[:], bg_d[l].rearrange("c p -> p c"))
                if use_bo:
                    bo_t = prm.tile([128, NC_D], F32, tag="bo")
                    nc.sync.dma_start(bo_t[:], bo_d[l].rearrange("c p -> p c"))

                # ---- LayerNorm rows chain (stats matmuls already accumulated into
                # ps_s/ps_q during the previous out-proj loop / preamble) ----
                mu = rows.tile([1, S], F32, tag="row")
                nc.vector.tensor_scalar_mul(mu[:], ps_s[:], 1.0 / D)
                musq = rows.tile([1, S], F32, tag="row")
                nc.vector.tensor_mul(musq[:], mu[:], mu[:])
                var = rows.tile([1, S], F32, tag="row")
                nc.vector.scalar_tensor_tensor(
                    var[:], ps_q[:], 1.0 / D, musq[:], op0=OP.mult, op1=OP.subtract
                )
                vare = rows.tile([1, S], F32, tag="row")
                nc.vector.tensor_scalar_add(vare[:], var[:], LN_EPS)
                sd = rows.tile([1, S], F32, tag="row")
                nc.scalar.activation(sd[:], vare[:], AF.Sqrt, bias=0.0, scale=1.0)
                rs = rows.tile([1, S], F32, tag="row")
                nc.vector.reciprocal(rs[:], sd[:])
                msr = rows.tile([1, S], F32, tag="row")
                nc.vector.tensor_mul(msr[:], mu[:], rs[:])
                # fp32 partition-broadcast on the (otherwise idle) GpSimd engine
                rs_b = bp_pool.tile([128, S], F32, tag="bb")
                nc.gpsimd.partition_broadcast(rs_b[:], rs[:])
                ms_b = bp_pool.tile([128, S], F32, tag="bb")
                nc.gpsimd.partition_broadcast(ms_b[:], msr[:])

                xn = xnp.tile([128, NC_D * S], DT, tag="xn")
                for c in range(NC_D):
                    t = tmp.tile([128, S], F32, tag="tmp2")
                    nc.vector.tensor_mul(t[:], x_cur[:, ts(c, S)], rs_b[:])
                    if use_lng or use_lnb:
                        t2 = tmp.tile([128, S], F32, tag="tmp2")
                        nc.vector.tensor_sub(t2[:], t[:], ms_b[:])
                        nc.scalar.activation(
                            xn[:, ts(c, S)], t2[:], AF.Identity,
                            bias=(lnb_t[:, c : c + 1] if use_lnb else 0.0),
                            scale=(lng_t[:, c : c + 1] if use_lng else 1.0),
                        )
                    else:
                        nc.vector.tensor_sub(xn[:, ts(c, S)], t[:], ms_b[:])

                # ---- V projection, token-major [t, fo] (xn stationary, wv moving) ----
                wv_t = wvp.tile([128, 16 * 512], DT, tag="wv")
                for i in range(16):
                    nc.sync.dma_start(wv_t[:, ts(i, 512)], wv_d[l, i])
                if use_bv:
                    vb = []
                    for foB in range(2):
                        pvb = psc.tile([128, 512], F32, tag="sc")
                        bvrow = rows.tile([1, 512], DT, tag="bvr")
                        nc.sync.dma_start(bvrow[:], bv_d[l, foB])
                        mm(pvb[:], oner[:], bvrow[:], start=True, stop=True)
                        vbt = bp_pool.tile([128, 512], F32, tag="vb")
                        nc.scalar.copy(vbt[:], pvb[:])
                        vb.append(vbt)
                v = vp.tile([128, NC_T * 1024], DT, tag="v")
                for foB in range(2):
                    for tc_ in range(NC_T):
                        pv = pmm.tile([128, 512], F32, tag="mm")
                        for fi in range(NC_D):
                            mm(
                                pv[:],
                                xn[:, fi * S + tc_ * 128 : fi * S + tc_ * 128 + 128],
                                wv_t[:, ts(foB * 8 + fi, 512)],
                                start=(fi == 0),
                                stop=(fi == NC_D - 1),
                            )
                        dst = v[:, tc_ * 1024 + foB * 512 : tc_ * 1024 + foB * 512 + 512]
                        if use_bv:
                            nc.vector.tensor_add(dst, pv[:], vb[foB][:])
                        else:
                            nc.vector.tensor_copy(dst, pv[:])

                # ---- gate: u = silu(xn @ gate_w + bg) (feature-major; u stays
                # fp32 — it only feeds the DVE gating multiply, never the PE) ----
                u = up.tile([128, NC_D * S], F32, tag="u")
                for f in range(NC_D):
                    wt = wbp.tile([128, 1024], DT, tag="wb")
                    nc.sync.dma_start(wt[:], wg_d[l, f])
                    pu = pmm.tile([128, S], F32, tag="mm")
                    for fi in range(NC_D):
                        mm(pu[:], wt[:, ts(fi, 128)], xn[:, ts(fi, S)],
                           start=(fi == 0), stop=(fi == NC_D - 1))
                    nc.scalar.activation(
                        u[:, ts(f, S)], pu[:], AF.Silu,
                        bias=(bg_t[:, f : f + 1] if use_bg else 0.0), scale=1.0,
                    )

                # ---- attention, one head pair (= one q/k feature chunk) at a time.
                # The two 64-wide heads run concurrently on the PE: scores via row
                # tiling (disjoint K row-groups), AV via col tiling into one
                # [128, S] PSUM tile. ----
                g = gp.tile([128, NC_D * S], DT, tag="g")
                for p_ in range(NP):
                    qc = qkp.tile([128, S], DT, tag="qc")
                    kc = qkp.tile([128, S], DT, tag="kc")
                    for (dst, fidx) in ((qc, p_), (kc, 8 + p_)):
                        wt = wbp.tile([128, 1024], DT, tag="wb")
                        nc.sync.dma_start(wt[:], wqk_d[l, fidx])
                        pq = pmm.tile([128, S], F32, tag="mm")
                        for fi in range(NC_D):
                            mm(pq[:], wt[:, ts(fi, 128)], xn[:, ts(fi, S)],
                               start=(fi == 0), stop=(fi == NC_D - 1))
                        if use_bqk:
                            nc.scalar.activation(
                                dst[:], pq[:], AF.Identity,
                                bias=bqk_t[:, fidx : fidx + 1], scale=1.0,
                            )
                        else:
                            nc.scalar.copy(dst[:], pq[:])

                    ao = pao.tile([128, S], F32, tag="ao")
                    for c in range(NC_T):
                        n = S - 128 * c
                        for (o, hh) in ((0, 2 * p_), (64, 2 * p_ + 1)):
                            sc = psc.tile([128, 512], F32, tag="sc")
                            mm(sc[:, :n], kc[o : o + 64, ts(c, 128)],
                               qc[o : o + 64, c * 128 : S], start=True, stop=True)
                            sig_bias = rpb_t[:, l * H + hh : l * H + hh + 1] if rpb_nz else 0.0
                            att = atp.tile([128, S], DT, tag="at")
                            dg = dtp.tile([128, 128], DT, tag="dg")
                            nc.scalar.activation(dg[:], sc[:, 0:128], AF.Sigmoid,
                                                 bias=sig_bias, scale=DH**-0.5)
                            nc.vector.tensor_mul(att[:, 0:128], dg[:], tri_t[:])
                            if n > 128:
                                nc.scalar.activation(att[:, 128:n], sc[:, 128:n],
                                                     AF.Sigmoid, bias=sig_bias,
                                                     scale=DH**-0.5)
                            mm(
                                ao[o : o + 64, c * 128 : S],
                                v[:, c * 1024 + p_ * 128 + o : c * 1024 + p_ * 128 + o + 64],
                                att[:, 0:n],
                                start=(c == 0),
                                stop=(c == NC_T - 1),
                                tile_position=(0, o),
                            )
                    nc.vector.tensor_mul(g[:, ts(p_, S)], ao[:], u[:, ts(p_, S)])

                # ---- out projection + residual + bf16 shadow refresh; next
                # layer's LN-stats matmuls accumulate here chunk-by-chunk so the
                # PE flows through the layer boundary without a stats stall ----
                x_new = xp.tile([128, NC_D * S], F32, tag="x")
                xb = xbp.tile([128, NC_D * S], DT, tag="xb")
                if l < L - 1:
                    ps_s = pst.tile([1, S], F32, tag="st")
                    ps_q = pst.tile([1, S], F32, tag="st")
                for f in range(NC_D):
                    wt = wbp.tile([128, 1024], DT, tag="wb")
                    nc.sync.dma_start(wt[:], wo_d[l, f])
                    pd = pmm.tile([128, S], F32, tag="mm")
                    for fi in range(NC_D):
                        mm(pd[:], wt[:, ts(fi, 128)], g[:, ts(fi, S)],
                           start=(fi == 0), stop=(fi == NC_D - 1))
                    if use_bo:
                        t3 = tmp.tile([128, S], F32, tag="tmp")
                        nc.scalar.activation(t3[:], pd[:], AF.Identity,
                                             bias=bo_t[:, f : f + 1], scale=1.0)
                        nc.vector.tensor_add(x_new[:, ts(f, S)], t3[:], x_cur[:, ts(f, S)])
                    else:
                        nc.vector.tensor_add(x_new[:, ts(f, S)], pd[:], x_cur[:, ts(f, S)])
                    nc.scalar.copy(xb[:, ts(f, S)], x_new[:, ts(f, S)])
                    if l < L - 1:
                        sq = tmp.tile([128, S], DT, tag="sq")
                        nc.vector.tensor_mul(sq[:], xb[:, ts(f, S)], xb[:, ts(f, S)])
                        mm(ps_s[:], onec[:], xb[:, ts(f, S)],
                           start=(f == 0), stop=(f == NC_D - 1))
                        mm(ps_q[:], onec[:], sq[:], start=(f == 0), stop=(f == NC_D - 1))
                x_cur = x_new

            # ---- logits: [V, T] feature(vocab)-major; xb is the bf16 final x ----
            for vo in range(NC_V):
                wt = wbp.tile([128, 1024], DT, tag="wb")
                nc.sync.dma_start(wt[:], wp_d[vo])
                pl = pmm.tile([128, S], F32, tag="mm")
                for fi in range(NC_D):
                    mm(pl[:], wt[:, ts(fi, 128)], xb[:, ts(fi, S)],
                       start=(fi == 0), stop=(fi == NC_D - 1))
                ot = op_pool.tile([128, S], F32, tag="o")
                if use_bp:
                    nc.scalar.activation(ot[:], pl[:], AF.Identity,
                                         bias=bp_t[:, vo : vo + 1], scale=1.0)
                elif vo % 2 == 0:
                    nc.scalar.copy(ot[:], pl[:])
                else:
                    nc.vector.tensor_copy(ot[:], pl[:])
                nc.sync.dma_start(out_d[vo], ot[:])

    nc.compile()
    return nc


def _get_program(cfg):
    nc = _prog_cache.get(cfg)
    if nc is None:
        nc = _build(cfg)
        _prog_cache[cfg] = nc
    return nc


def _marshal(inputs):
    """Host-side input marshalling into DMA-friendly bf16 layouts."""
    f = np.float32
    input_ids = np.asarray(inputs["input_ids"])
    emb = np.asarray(inputs["embedding"], f)
    pos = np.asarray(inputs["pos_encoding"], f)
    qkv_w = np.asarray(inputs["qkv_w"], f)
    gate_w = np.asarray(inputs["gate_w"], f)
    out_w = np.asarray(inputs["out_w"], f)
    proj_w = np.asarray(inputs["proj_w"], f)

    x0 = emb[input_ids] + pos[:, :S, :]                       # [B, S, D]
    # feature-major per core: [D, S] -> [NC_D, 128, S]
    x0t = np.ascontiguousarray(x0.transpose(0, 2, 1)).reshape(B, NC_D, 128, S)

    # lhsT tiles [K=fi(128), M=fo(128)] packed 8-fi-wide: [l, fo, 128p, 8c*128j]
    def lhs_tiles(w, nfo):  # w: [L, D, nfo*128]
        r = w.reshape(L, NC_D, 128, nfo, 128).transpose(0, 3, 2, 1, 4)
        return np.ascontiguousarray(r.reshape(L, nfo, 128, NC_D * 128)).astype(BF16_NP)

    wqk = lhs_tiles(qkv_w[:, :, :2048], 16)                   # [6,16,128,1024]
    wg = lhs_tiles(gate_w, NC_D)                              # [6,8,128,1024]
    wo = lhs_tiles(out_w, NC_D)                               # [6,8,128,1024]
    # v-section as moving tiles [K=fi(128), N=fo(512)]: [l, foB*8+fi, 128, 512]
    wv = qkv_w[:, :, 2048:].reshape(L, NC_D, 128, 2, 512).transpose(0, 3, 1, 2, 4)
    wv = np.ascontiguousarray(wv.reshape(L, 16, 128, 512)).astype(BF16_NP)
    wp = proj_w.reshape(NC_D, 128, NC_V, 128).transpose(2, 1, 0, 3)
    wp = np.ascontiguousarray(wp.reshape(NC_V, 128, NC_D * 128)).astype(BF16_NP)

    tri = np.triu(np.ones((128, 128), f)).astype(BF16_NP)
    onec = np.ones((128, 1), BF16_NP)
    oner = np.ones((1, 128), BF16_NP)

    qkv_b = np.asarray(inputs["qkv_b"], f)
    gate_b = np.asarray(inputs["gate_b"], f)
    out_b = np.asarray(inputs["out_b"], f)
    proj_b = np.asarray(inputs["proj_b"], f)
    ln_g = np.asarray(inputs["ln_g"], f)
    ln_b = np.asarray(inputs["ln_b"], f)
    rpb = np.asarray(inputs["rel_pos_bias"], f)

    use_lng = not np.all(ln_g == 1.0)
    use_lnb = np.any(ln_b != 0.0)
    use_bqk = np.any(qkv_b[:, :2048] != 0.0)
    use_bv = np.any(qkv_b[:, 2048:] != 0.0)
    use_bg = np.any(gate_b != 0.0)
    use_bo = np.any(out_b != 0.0)
    use_bp = np.any(proj_b != 0.0)
    rpb_nz = bool(np.any(rpb != 0.0))

    shared = {
        "wqk": wqk, "wv": wv, "wg": wg, "wo": wo, "wp": wp,
        "tri": tri, "onec": onec, "oner": oner,
    }
    if use_lng:
        shared["lng"] = np.ascontiguousarray(ln_g.reshape(L, NC_D, 128))
    if use_lnb:
        shared["lnb"] = np.ascontiguousarray(ln_b.reshape(L, NC_D, 128))
    if use_bqk:
        shared["bqk"] = np.ascontiguousarray(qkv_b[:, :2048].reshape(L, 16, 128))
    if use_bv:
        shared["bv"] = np.ascontiguousarray(qkv_b[:, 2048:].reshape(L, 2, 1, 512)).astype(BF16_NP)
    if use_bg:
        shared["bg"] = np.ascontiguousarray(gate_b.reshape(L, NC_D, 128))
    if use_bo:
        shared["bo"] = np.ascontiguousarray(out_b.reshape(L, NC_D, 128))
    if use_bp:
        shared["bp"] = np.ascontiguousarray(proj_b.reshape(NC_V, 128))
    if rpb_nz:
        shared["rpb"] = np.ascontiguousarray(rpb.reshape(1, L * H)).astype(BF16_NP)

    cfg = (use_lng, use_lnb, use_bqk, use_bv, use_bg, use_bo, use_bp, rpb_nz)
    in_maps = []
    for b in range(B):
        m = dict(shared)
        m["x0t"] = np.ascontiguousarray(x0t[b])
        in_maps.append(m)
    return cfg, in_maps


def run(inputs, mm_mode="bf16", trace=False):
    cfg, in_maps = _marshal(inputs)
    nc = _get_program(cfg)
    res = run_bass_kernel_spmd(nc, in_maps, core_ids=list(range(N_CORES)), trace=trace)
    out = np.empty((B, S, V), np.float32)
    for b in range(B):
        lt = res.results[b]["logits_t"].reshape(V, S)
        out[b] = lt.T
    return out, res


def kernel(**inputs) -> np.ndarray:
    out, _ = run(inputs, trace=False)
    return out


# revision 15
# speedup vs baseline: 1.1044x; 1.1044x over previous
"""HSTU-style dense transformer for sequence modeling on 8 Trainium2 NeuronCores.

Sharding: data-parallel over batch (B=8 -> 1 sequence per core). All weights
replicated. Activations are kept feature-major [D=partitions, T=free] on chip;
attention computes transposed scores [kt, qt] so the sigmoid/causal-trim/AV
chain needs no on-device transposes.

All matmul operands are bf16 (weights pre-cast on host): bf16 gets the fast
weight-load path so LDWEIGHTS hides under the previous matmul's streaming, and
weight DMA traffic halves vs f32. The residual stream x and all PSUM
accumulation stay fp32; a bf16 shadow copy of x (xb) feeds every GEMM.

LayerNorm is DEFERRED past the GEMMs: since
    W @ ((x - mu) * rs) = rs * (W @ x) - (rs * mu) * colsum(W),
the qkv/gate projections run directly on raw xb (available the moment the
previous layer's residual lands, so the PE never idles at a layer boundary)
and the per-token factors rs / mu*rs are applied at PSUM evacuation as two
vector ops, using host-precomputed negated column sums. The factor chain
(mean/var/sqrt/reciprocal rows + GpSimd partition-broadcasts + a DRAM-
roundtrip transpose for the token-major V factors) runs concurrently with the
GEMM stream instead of serializing in front of it. The LN affine (ln_g/ln_b)
is absorbed into the projection weights and biases on the host.

Host side only marshals: embedding gather + positional add, weight pre-tiling
into DMA-contiguous bf16 layouts, colsum precompute, and the final
[V,T] -> [S,V] untranspose.
"""

import sys

sys.path.insert(0, "/opt/trn_rl_repo")

import numpy as np

import concourse.bass as bass  # noqa: F401  (keeps bass registered before bacc)
import concourse.tile as tile
from concourse import bacc, mybir
from concourse.bass import ts
from concourse.bass_utils import run_bass_kernel_spmd

B, S, D, H, L, V = 8, 512, 1024, 16, 6, 32000
DH = D // H
LN_EPS = 1e-5
N_CORES = 8
NC_D = D // 128      # 8 feature chunks
NC_T = S // 128      # 4 token chunks
NC_V = V // 128      # 250 vocab chunks
NP = 8               # head pairs

F32 = mybir.dt.float32
BF16 = mybir.dt.bfloat16
BF16_NP = mybir.dt.np(mybir.dt.bfloat16)
AF = mybir.ActivationFunctionType
OP = mybir.AluOpType

_prog_cache = {}


def _build(cfg):
    """Build + compile the SPMD per-core program. cfg is a hashable dict-key."""
    (use_bqk, use_bv, use_bg, use_bo, use_bp, rpb_nz) = cfg
    DT = BF16

    nc = bacc.Bacc("TRN2", target_bir_lowering=False, debug=False)

    x0_d = nc.dram_tensor("x0t", [NC_D, 128, S], F32, kind="ExternalInput").ap()
    wqk_d = nc.dram_tensor("wqk", [L, 16, 128, 1024], DT, kind="ExternalInput").ap()
    wv_d = nc.dram_tensor("wv", [L, 16, 128, 512], DT, kind="ExternalInput").ap()
    wg_d = nc.dram_tensor("wg", [L, NC_D, 128, 1024], DT, kind="ExternalInput").ap()
    wo_d = nc.dram_tensor("wo", [L, NC_D, 128, 1024], DT, kind="ExternalInput").ap()
    wp_d = nc.dram_tensor("wp", [NC_V, 128, 1024], DT, kind="ExternalInput").ap()
    ncs_qk_d = nc.dram_tensor("ncs_qk", [L, 16, 128], F32, kind="ExternalInput").ap()
    ncs_g_d = nc.dram_tensor("ncs_g", [L, NC_D, 128], F32, kind="ExternalInput").ap()
    ncs_v_d = nc.dram_tensor("ncs_v", [L, 128, 1024], F32, kind="ExternalInput").ap()
    tri_d = nc.dram_tensor("tri", [128, 128], DT, kind="ExternalInput").ap()
    onec_d = nc.dram_tensor("onec", [128, 1], DT, kind="ExternalInput").ap()
    bqk_d = nc.dram_tensor("bqk", [L, 16, 128], F32, kind="ExternalInput").ap() if use_bqk else None
    bv_d = nc.dram_tensor("bv", [L, 2, 1, 512], F32, kind="ExternalInput").ap() if use_bv else None
    bg_d = nc.dram_tensor("bg", [L, NC_D, 128], F32, kind="ExternalInput").ap() if use_bg else None
    bo_d = nc.dram_tensor("bo", [L, NC_D, 128], F32, kind="ExternalInput").ap() if use_bo else None
    bp_d = nc.dram_tensor("bp", [NC_V, 128], F32, kind="ExternalInput").ap() if use_bp else None
    rpb_d = nc.dram_tensor("rpb", [1, H * L], F32, kind="ExternalInput").ap() if rpb_nz else None
    # scratch for the [1,S] -> [128, NC_T] row->column transpose round-trip
    rm_d = nc.dram_tensor("rm_scratch", [L, 2, 512], F32, kind="Internal").ap()
    out_d = nc.dram_tensor("logits_t", [NC_V, 128, S], F32, kind="ExternalOutput").ap()

    with tile.TileContext(nc) as tc, nc.allow_low_precision(
        reason="bf16 tiles feed the PE; accumulation stays fp32 in PSUM"
    ):
        from contextlib import ExitStack

        with ExitStack() as ctx:
            cp = ctx.enter_context(tc.tile_pool(name="consts", bufs=1))
            xp = ctx.enter_context(tc.tile_pool(name="x", bufs=2))
            xbp = ctx.enter_context(tc.tile_pool(name="xb", bufs=1))
            up = ctx.enter_context(tc.tile_pool(name="u", bufs=1))
            vp = ctx.enter_context(tc.tile_pool(name="v", bufs=1))
            gp = ctx.enter_context(tc.tile_pool(name="g", bufs=1))
            qkp = ctx.enter_context(tc.tile_pool(name="qk", bufs=2))
            tmp = ctx.enter_context(tc.tile_pool(name="tmp", bufs=3))
            rows = ctx.enter_context(tc.tile_pool(name="rows", bufs=4))
            bp_pool = ctx.enter_context(tc.tile_pool(name="bcast", bufs=4))
            atp = ctx.enter_context(tc.tile_pool(name="at", bufs=4))
            wbp = ctx.enter_context(tc.tile_pool(name="wb", bufs=4))
            wvp = ctx.enter_context(tc.tile_pool(name="wvp", bufs=1))
            op_pool = ctx.enter_context(tc.tile_pool(name="out", bufs=4))
            prm = ctx.enter_context(tc.tile_pool(name="prm", bufs=2))
            pmm = ctx.enter_context(tc.tile_pool(name="pmm", bufs=2, space="PSUM"))
            pao = ctx.enter_context(tc.tile_pool(name="pao", bufs=1, space="PSUM"))
            psc = ctx.enter_context(tc.tile_pool(name="psc", bufs=3, space="PSUM"))
            pst = ctx.enter_context(tc.tile_pool(name="pst", bufs=2, space="PSUM"))

            mm = nc.tensor.matmul

            onec = cp.tile([128, 1], DT)
            nc.sync.dma_start(onec[:], onec_d[:])
            tri_t = cp.tile([128, 128], DT)
            nc.sync.dma_start(tri_t[:], tri_d[:])
            if rpb_nz:
                rpb_row = cp.tile([1, H * L], F32)
                nc.sync.dma_start(rpb_row[:], rpb_d[:])
                rpb_t = cp.tile([128, H * L], F32)
                nc.gpsimd.partition_broadcast(rpb_t[:], rpb_row[:])

            x_cur = xp.tile([128, NC_D * S], F32, tag="x")
            for c in range(NC_D):
                nc.sync.dma_start(x_cur[:, ts(c, S)], x0_d[c])
            # bf16 shadow of x: feeds every GEMM, the LN-stats matmuls, and the
            # final logits GEMM
            xb = xbp.tile([128, NC_D * S], DT, tag="xb")
            ps_s = pst.tile([1, S], F32, tag="st")
            ps_q = pst.tile([1, S], F32, tag="st")
            for c in range(NC_D):
                nc.scalar.copy(xb[:, ts(c, S)], x_cur[:, ts(c, S)])
                sq = tmp.tile([128, S], DT, tag="sq")
                nc.vector.tensor_mul(sq[:], xb[:, ts(c, S)], xb[:, ts(c, S)])
                mm(ps_s[:], onec[:], xb[:, ts(c, S)], start=(c == 0), stop=(c == NC_D - 1))
                mm(ps_q[:], onec[:], sq[:], start=(c == 0), stop=(c == NC_D - 1))

            for l in range(L):
                # ---- per-layer params ----
                ncs_qk = prm.tile([128, 16], F32, tag="ncsqk")
                nc.sync.dma_start(ncs_qk[:], ncs_qk_d[l].rearrange("c p -> p c"))
                ncs_g = prm.tile([128, NC_D], F32, tag="ncsg")
                nc.sync.dma_start(ncs_g[:], ncs_g_d[l].rearrange("c p -> p c"))
                ncs_v = prm.tile([128, 1024], F32, tag="ncsv")
                nc.sync.dma_start(ncs_v[:], ncs_v_d[l])
                if use_bqk:
                    bqk_t = prm.tile([128, 16], F32, tag="bqk")
                    nc.sync.dma_start(bqk_t[:], bqk_d[l].rearrange("c p -> p c"))
                if use_bg:
                    bg_t = prm.tile([128, NC_D], F32, tag="bg")
                    nc.sync.dma_start(bg_t[:], bg_d[l].rearrange("c p -> p c"))
                if use_bo:
                    bo_t = prm.tile([128, NC_D], F32, tag="bo")
                    nc.sync.dma_start(bo_t[:], bo_d[l].rearrange("c p -> p c"))

                # ---- LN factor chain (overlaps the GEMM stream below; nothing
                # on the PE waits for it until first evacuation) ----
                mu = rows.tile([1, S], F32, tag="row")
                nc.vector.tensor_scalar_mul(mu[:], ps_s[:], 1.0 / D)
                musq = rows.tile([1, S], F32, tag="row")
                nc.vector.tensor_mul(musq[:], mu[:], mu[:])
                var = rows.tile([1, S], F32, tag="row")
                nc.vector.scalar_tensor_tensor(
                    var[:], ps_q[:], 1.0 / D, musq[:], op0=OP.mult, op1=OP.subtract
                )
                vare = rows.tile([1, S], F32, tag="row")
                nc.vector.tensor_scalar_add(vare[:], var[:], LN_EPS)
                sd = rows.tile([1, S], F32, tag="row")
                nc.scalar.activation(sd[:], vare[:], AF.Sqrt, bias=0.0, scale=1.0)
                rs = rows.tile([1, S], F32, tag="row")
                nc.vector.reciprocal(rs[:], sd[:])
                # row-broadcast [1,S] -> [128,S] on the idle GpSimd engine
                rs_b = bp_pool.tile([128, S], F32, tag="bb")
                nc.gpsimd.partition_broadcast(rs_b[:], rs[:])
                mu_b = bp_pool.tile([128, S], F32, tag="bb")
                nc.gpsimd.partition_broadcast(mu_b[:], mu[:])
                # column orientation [128 tok, NC_T] via DRAM round-trip
                nc.sync.dma_start(rm_d[l, 0:1], rs[:])
                nc.sync.dma_start(rm_d[l, 1:2], mu[:])
                rs_c = prm.tile([128, NC_T], F32, tag="rsc")
                nc.sync.dma_start(
                    rs_c[:], rm_d[l, 0].rearrange("(c p) -> p c", p=128))
                mu_c = prm.tile([128, NC_T], F32, tag="msc")
                nc.sync.dma_start(
                    mu_c[:], rm_d[l, 1].rearrange("(c p) -> p c", p=128))
                if use_bv:
                    vb = []
                    for foB in range(2):
                        bvrow = rows.tile([1, 512], F32, tag="bvr")
                        nc.sync.dma_start(bvrow[:], bv_d[l, foB])
                        vbt = bp_pool.tile([128, 512], F32, tag="vb")
                        nc.gpsimd.partition_broadcast(vbt[:], bvrow[:])
                        vb.append(vbt)

                # ---- V projection, token-major [t, fo] (xb stationary, wv
                # moving); raw evacuation, LN factors applied in place once the
                # factor chain lands ----
                wv_t = wvp.tile([128, 16 * 512], DT, tag="wv")
                for i in range(16):
                    nc.sync.dma_start(wv_t[:, ts(i, 512)], wv_d[l, i])
                v = vp.tile([128, NC_T * 1024], DT, tag="v")
                for foB in range(2):
                    for tc_ in range(NC_T):
                        pv = pmm.tile([128, 512], F32, tag="mm")
                        for fi in range(NC_D):
                            mm(
                                pv[:],
                                xb[:, fi * S + tc_ * 128 : fi * S + tc_ * 128 + 128],
                                wv_t[:, ts(foB * 8 + fi, 512)],
                                start=(fi == 0),
                                stop=(fi == NC_D - 1),
                            )
                        dst = v[:, tc_ * 1024 + foB * 512 : tc_ * 1024 + foB * 512 + 512]
                        nc.scalar.copy(dst, pv[:])
                        nc.vector.scalar_tensor_tensor(
                            dst, ncs_v[:, ts(foB, 512)], mu_c[:, tc_ : tc_ + 1],
                            dst, op0=OP.mult, op1=OP.add,
                        )
                        nc.vector.tensor_scalar_mul(dst, dst, rs_c[:, tc_ : tc_ + 1])
                        if use_bv:
                            nc.vector.tensor_add(dst, dst, vb[foB][:])

                # ---- gate: u = silu(rs*(xb @ gate_w - mu*colsum) + bg); u
                # stays fp32 — it only feeds the DVE gating multiply ----
                u = up.tile([128, NC_D * S], F32, tag="u")
                for f in range(NC_D):
                    wt = wbp.tile([128, 1024], DT, tag="wb")
                    nc.sync.dma_start(wt[:], wg_d[l, f])
                    pu = pmm.tile([128, S], F32, tag="mm")
                    for fi in range(NC_D):
                        mm(pu[:], wt[:, ts(fi, 128)], xb[:, ts(fi, S)],
                           start=(fi == 0), stop=(fi == NC_D - 1))
                    us = u[:, ts(f, S)]
                    nc.vector.scalar_tensor_tensor(
                        us, mu_b[:], ncs_g[:, f : f + 1], pu[:],
                        op0=OP.mult, op1=OP.add,
                    )
                    nc.vector.tensor_mul(us, us, rs_b[:])
                    nc.scalar.activation(
                        us, us, AF.Silu,
                        bias=(bg_t[:, f : f + 1] if use_bg else 0.0), scale=1.0,
                    )

                # ---- attention, one head pair (= one q/k feature chunk) at a
                # time. Scores for the two 64-wide heads issue back-to-back
                # (disjoint PE row groups), AV back-to-back via col tiling into
                # one [128, S] PSUM tile. ----
                g = gp.tile([128, NC_D * S], DT, tag="g")
                for p_ in range(NP):
                    qc = qkp.tile([128, S], DT, tag="qc")
                    kc = qkp.tile([128, S], DT, tag="kc")
                    for (dst, fidx) in ((qc, p_), (kc, 8 + p_)):
                        wt = wbp.tile([128, 1024], DT, tag="wb")
                        nc.sync.dma_start(wt[:], wqk_d[l, fidx])
                        pq = pmm.tile([128, S], F32, tag="mm")
                        for fi in range(NC_D):
                            mm(pq[:], wt[:, ts(fi, 128)], xb[:, ts(fi, S)],
                               start=(fi == 0), stop=(fi == NC_D - 1))
                        nc.vector.scalar_tensor_tensor(
                            dst[:], mu_b[:], ncs_qk[:, fidx : fidx + 1], pq[:],
                            op0=OP.mult, op1=OP.add,
                        )
                        nc.vector.tensor_mul(dst[:], dst[:], rs_b[:])
                        if use_bqk:
                            nc.vector.tensor_scalar_add(
                                dst[:], dst[:], bqk_t[:, fidx : fidx + 1])

                    ao = pao.tile([128, S], F32, tag="ao")
                    for c in range(NC_T):
                        n = S - 128 * c
                        sc_e = psc.tile([128, 512], F32, tag="sc")
                        sc_o = psc.tile([128, 512], F32, tag="sc")
                        mm(sc_e[:, :n], kc[0:64, ts(c, 128)],
                           qc[0:64, c * 128 : S], start=True, stop=True)
                        mm(sc_o[:, :n], kc[64:128, ts(c, 128)],
                           qc[64:128, c * 128 : S], start=True, stop=True)
                        att = {}
                        for (o, hh, sc) in ((0, 2 * p_, sc_e), (64, 2 * p_ + 1, sc_o)):
                            sig_bias = rpb_t[:, l * H + hh : l * H + hh + 1] if rpb_nz else 0.0
                            at = atp.tile([128, S], DT, tag="at")
                            nc.scalar.activation(at[:, 0:n], sc[:, 0:n], AF.Sigmoid,
                                                 bias=sig_bias, scale=DH**-0.5)
                            nc.vector.tensor_mul(at[:, 0:128], at[:, 0:128], tri_t[:])
                            att[o] = at
                        for o in (0, 64):
                            mm(
                                ao[o : o + 64, c * 128 : S],
                                v[:, c * 1024 + p_ * 128 + o : c * 1024 + p_ * 128 + o + 64],
                                att[o][:, 0:n],
                                start=(c == 0),
                                stop=(c == NC_T - 1),
                                tile_position=(0, o),
                            )
                    nc.vector.tensor_mul(g[:, ts(p_, S)], ao[:], u[:, ts(p_, S)])

                # ---- out projection + residual + bf16 shadow refresh; next
                # layer's LN-stats matmuls accumulate here chunk-by-chunk so the
                # PE flows through the layer boundary without a stats stall ----
                x_new = xp.tile([128, NC_D * S], F32, tag="x")
                xb = xbp.tile([128, NC_D * S], DT, tag="xb")
                if l < L - 1:
                    ps_s = pst.tile([1, S], F32, tag="st")
                    ps_q = pst.tile([1, S], F32, tag="st")
                for f in range(NC_D):
                    wt = wbp.tile([128, 1024], DT, tag="wb")
                    nc.sync.dma_start(wt[:], wo_d[l, f])
                    pd = pmm.tile([128, S], F32, tag="mm")
                    for fi in range(NC_D):
                        mm(pd[:], wt[:, ts(fi, 128)], g[:, ts(fi, S)],
                           start=(fi == 0), stop=(fi == NC_D - 1))
                    if use_bo:
                        t3 = tmp.tile([128, S], F32, tag="tmp")
                        nc.scalar.activation(t3[:], pd[:], AF.Identity,
                                             bias=bo_t[:, f : f + 1], scale=1.0)
                        nc.vector.tensor_add(x_new[:, ts(f, S)], t3[:], x_cur[:, ts(f, S)])
                    else:
                        nc.vector.tensor_add(x_new[:, ts(f, S)], pd[:], x_cur[:, ts(f, S)])
                    nc.scalar.copy(xb[:, ts(f, S)], x_new[:, ts(f, S)])
                    if l < L - 1:
                        sq = tmp.tile([128, S], DT, tag="sq")
                        nc.vector.tensor_mul(sq[:], xb[:, ts(f, S)], xb[:, ts(f, S)])
                        mm(ps_s[:], onec[:], xb[:, ts(f, S)],
                           start=(f == 0), stop=(f == NC_D - 1))
                        mm(ps_q[:], onec[:], sq[:], start=(f == 0), stop=(f == NC_D - 1))
                x_cur = x_new

            # ---- logits: [V, T] feature(vocab)-major; xb is the bf16 final x ----
            if use_bp:
                bp_t = cp.tile([128, NC_V], F32)
                nc.sync.dma_start(bp_t[:], bp_d.rearrange("v p -> p v"))
            for vo in range(NC_V):
                wt = wbp.tile([128, 1024], DT, tag="wb")
                nc.sync.dma_start(wt[:], wp_d[vo])
                pl = pmm.tile([128, S], F32, tag="mm")
                for fi in range(NC_D):
                    mm(pl[:], wt[:, ts(fi, 128)], xb[:, ts(fi, S)],
                       start=(fi == 0), stop=(fi == NC_D - 1))
                ot = op_pool.tile([128, S], F32, tag="o")
                if use_bp:
                    nc.scalar.activation(ot[:], pl[:], AF.Identity,
                                         bias=bp_t[:, vo : vo + 1], scale=1.0)
                elif vo % 2 == 0:
                    nc.scalar.copy(ot[:], pl[:])
                else:
                    nc.vector.tensor_copy(ot[:], pl[:])
                nc.sync.dma_start(out_d[vo], ot[:])

    nc.compile()
    return nc


def _get_program(cfg):
    nc = _prog_cache.get(cfg)
    if nc is None:
        nc = _build(cfg)
        _prog_cache[cfg] = nc
    return nc


def _marshal(inputs):
    """Host-side input marshalling into DMA-friendly bf16 layouts."""
    f = np.float32
    input_ids = np.asarray(inputs["input_ids"])
    emb = np.asarray(inputs["embedding"], f)
    pos = np.asarray(inputs["pos_encoding"], f)
    qkv_w = np.asarray(inputs["qkv_w"], f)
    gate_w = np.asarray(inputs["gate_w"], f)
    out_w = np.asarray(inputs["out_w"], f)
    proj_w = np.asarray(inputs["proj_w"], f)
    qkv_b = np.asarray(inputs["qkv_b"], f)
    gate_b = np.asarray(inputs["gate_b"], f)
    out_b = np.asarray(inputs["out_b"], f)
    proj_b = np.asarray(inputs["proj_b"], f)
    ln_g = np.asarray(inputs["ln_g"], f)
    ln_b = np.asarray(inputs["ln_b"], f)
    rpb = np.asarray(inputs["rel_pos_bias"], f)

    x0 = emb[input_ids] + pos[:, :S, :]                       # [B, S, D]
    # feature-major per core: [D, S] -> [NC_D, 128, S]
    x0t = np.ascontiguousarray(x0.transpose(0, 2, 1)).reshape(B, NC_D, 128, S)

    # absorb the LN affine into the LN-consuming projections:
    #   W_eff[f,o] = W[f,o] * ln_g[f];  b_eff[o] = b[o] + ln_b @ W
    qkv_w_eff = qkv_w * ln_g[:, :, None]
    qkv_b_eff = qkv_b + np.einsum("lf,lfo->lo", ln_b, qkv_w)
    gate_w_eff = gate_w * ln_g[:, :, None]
    gate_b_eff = gate_b + np.einsum("lf,lfo->lo", ln_b, gate_w)

    # negated column sums (for the deferred mean subtraction)
    ncs_qk = -qkv_w_eff[:, :, :2048].sum(axis=1)              # [L, 2048]
    ncs_g = -gate_w_eff.sum(axis=1)                           # [L, 1024]
    ncs_v_row = -qkv_w_eff[:, :, 2048:].sum(axis=1)           # [L, 1024]
    ncs_v = np.ascontiguousarray(
        np.broadcast_to(ncs_v_row[:, None, :], (L, 128, 1024)))

    # lhsT tiles [K=fi(128), M=fo(128)] packed 8-fi-wide: [l, fo, 128p, 8c*128j]
    def lhs_tiles(w, nfo):  # w: [L, D, nfo*128]
        r = w.reshape(L, NC_D, 128, nfo, 128).transpose(0, 3, 2, 1, 4)
        return np.ascontiguousarray(r.reshape(L, nfo, 128, NC_D * 128)).astype(BF16_NP)

    wqk = lhs_tiles(qkv_w_eff[:, :, :2048], 16)               # [6,16,128,1024]
    wg = lhs_tiles(gate_w_eff, NC_D)                          # [6,8,128,1024]
    wo = lhs_tiles(out_w, NC_D)                               # [6,8,128,1024]
    # v-section as moving tiles [K=fi(128), N=fo(512)]: [l, foB*8+fi, 128, 512]
    wv = qkv_w_eff[:, :, 2048:].reshape(L, NC_D, 128, 2, 512).transpose(0, 3, 1, 2, 4)
    wv = np.ascontiguousarray(wv.reshape(L, 16, 128, 512)).astype(BF16_NP)
    wp = proj_w.reshape(NC_D, 128, NC_V, 128).transpose(2, 1, 0, 3)
    wp = np.ascontiguousarray(wp.reshape(NC_V, 128, NC_D * 128)).astype(BF16_NP)

    tri = np.triu(np.ones((128, 128), f)).astype(BF16_NP)
    onec = np.ones((128, 1), BF16_NP)

    use_bqk = np.any(qkv_b_eff[:, :2048] != 0.0)
    use_bv = np.any(qkv_b_eff[:, 2048:] != 0.0)
    use_bg = np.any(gate_b_eff != 0.0)
    use_bo = np.any(out_b != 0.0)
    use_bp = np.any(proj_b != 0.0)
    rpb_nz = bool(np.any(rpb != 0.0))

    shared = {
        "wqk": wqk, "wv": wv, "wg": wg, "wo": wo, "wp": wp,
        "ncs_qk": np.ascontiguousarray(ncs_qk.reshape(L, 16, 128)),
        "ncs_g": np.ascontiguousarray(ncs_g.reshape(L, NC_D, 128)),
        "ncs_v": ncs_v,
        "tri": tri, "onec": onec,
    }
    if use_bqk:
        shared["bqk"] = np.ascontiguousarray(qkv_b_eff[:, :2048].reshape(L, 16, 128))
    if use_bv:
        shared["bv"] = np.ascontiguousarray(qkv_b_eff[:, 2048:].reshape(L, 2, 1, 512))
    if use_bg:
        shared["bg"] = np.ascontiguousarray(gate_b_eff.reshape(L, NC_D, 128))
    if use_bo:
        shared["bo"] = np.ascontiguousarray(out_b.reshape(L, NC_D, 128))
    if use_bp:
        shared["bp"] = np.ascontiguousarray(proj_b.reshape(NC_V, 128))
    if rpb_nz:
        shared["rpb"] = np.ascontiguousarray(rpb.reshape(1, L * H))

    cfg = (use_bqk, use_bv, use_bg, use_bo, use_bp, rpb_nz)
    in_maps = []
    for b in range(B):
        m = dict(shared)
        m["x0t"] = np.ascontiguousarray(x0t[b])
        in_maps.append(m)
    return cfg, in_maps


def run(inputs, mm_mode="bf16", trace=False):
    cfg, in_maps = _marshal(inputs)
    nc = _get_program(cfg)
    res = run_bass_kernel_spmd(nc, in_maps, core_ids=list(range(N_CORES)), trace=trace)
    out = np.empty((B, S, V), np.float32)
    for b in range(B):
        lt = res.results[b]["logits_t"].reshape(V, S)
        out[b] = lt.T
    return out, res


def kernel(**inputs) -> np.ndarray:
    out, _ = run(inputs, trace=False)
    return out


# revision 18
# speedup vs baseline: 1.1247x; 1.0184x over previous
"""HSTU-style dense transformer for sequence modeling on 8 Trainium2 NeuronCores.

Sharding: data-parallel over batch (B=8 -> 1 sequence per core). All weights
replicated. Activations are kept feature-major [D=partitions, T=free] on chip;
attention computes transposed scores [kt, qt] so the sigmoid/causal-trim/AV
chain needs no on-device transposes.

All matmul operands are bf16 (weights pre-cast on host): bf16 gets the fast
weight-load path so LDWEIGHTS hides under the previous matmul's streaming, and
weight DMA traffic halves vs f32. The residual stream x and all PSUM
accumulation stay fp32; a bf16 shadow copy of x (xb) feeds every GEMM.

LayerNorm is DEFERRED past the GEMMs: since
    W @ ((x - mu) * rs) = rs * (W @ x) - (rs * mu) * colsum(W),
the qkv/gate projections run directly on raw xb (available the moment the
previous layer's residual lands, so the PE never idles at a layer boundary)
and the per-token factors rs / mu*rs are applied at PSUM evacuation as two
vector ops, using host-precomputed negated column sums. The factor chain
(mean/var/sqrt/reciprocal rows + GpSimd partition-broadcasts + a DRAM-
roundtrip transpose for the token-major V factors) runs concurrently with the
GEMM stream instead of serializing in front of it. The LN affine (ln_g/ln_b)
is absorbed into the projection weights and biases on the host.

Host side only marshals: embedding gather + positional add, weight pre-tiling
into DMA-contiguous bf16 layouts, colsum precompute, and the final
[V,T] -> [S,V] untranspose.
"""

import sys

sys.path.insert(0, "/opt/trn_rl_repo")

import numpy as np

import concourse.bass as bass  # noqa: F401  (keeps bass registered before bacc)
import concourse.tile as tile
from concourse import bacc, mybir
from concourse.bass import ts
from concourse.bass_utils import run_bass_kernel_spmd

B, S, D, H, L, V = 8, 512, 1024, 16, 6, 32000
DH = D // H
LN_EPS = 1e-5
N_CORES = 8
NC_D = D // 128      # 8 feature chunks
NC_T = S // 128      # 4 token chunks
NC_V = V // 128      # 250 vocab chunks
NP = 8               # head pairs

F32 = mybir.dt.float32
BF16 = mybir.dt.bfloat16
BF16_NP = mybir.dt.np(mybir.dt.bfloat16)
AF = mybir.ActivationFunctionType
OP = mybir.AluOpType

_prog_cache = {}


def _build(cfg):
    """Build + compile the SPMD per-core program. cfg is a hashable dict-key."""
    (use_bqk, use_bv, use_bg, use_bo, use_bp, rpb_nz) = cfg
    DT = BF16

    nc = bacc.Bacc("TRN2", target_bir_lowering=False, debug=False)

    x0_d = nc.dram_tensor("x0t", [NC_D, 128, S], F32, kind="ExternalInput").ap()
    wqk_d = nc.dram_tensor("wqk", [L, 16, 128, 1024], DT, kind="ExternalInput").ap()
    wv_d = nc.dram_tensor("wv", [L, 16, 128, 512], DT, kind="ExternalInput").ap()
    wg_d = nc.dram_tensor("wg", [L, NC_D, 128, 1024], DT, kind="ExternalInput").ap()
    wo_d = nc.dram_tensor("wo", [L, NC_D, 128, 1024], DT, kind="ExternalInput").ap()
    wp_d = nc.dram_tensor("wp", [NC_V, 128, 1024], DT, kind="ExternalInput").ap()
    ncs_qk_d = nc.dram_tensor("ncs_qk", [L, 16, 128], F32, kind="ExternalInput").ap()
    ncs_g_d = nc.dram_tensor("ncs_g", [L, NC_D, 128], F32, kind="ExternalInput").ap()
    ncs_v_d = nc.dram_tensor("ncs_v", [L, 128, 1024], F32, kind="ExternalInput").ap()
    tri_d = nc.dram_tensor("tri", [128, 128], DT, kind="ExternalInput").ap()
    onec_d = nc.dram_tensor("onec", [128, 1], DT, kind="ExternalInput").ap()
    bqk_d = nc.dram_tensor("bqk", [L, 16, 128], F32, kind="ExternalInput").ap() if use_bqk else None
    bv_d = nc.dram_tensor("bv", [L, 2, 1, 512], F32, kind="ExternalInput").ap() if use_bv else None
    bg_d = nc.dram_tensor("bg", [L, NC_D, 128], F32, kind="ExternalInput").ap() if use_bg else None
    bo_d = nc.dram_tensor("bo", [L, NC_D, 128], F32, kind="ExternalInput").ap() if use_bo else None
    bp_d = nc.dram_tensor("bp", [NC_V, 128], F32, kind="ExternalInput").ap() if use_bp else None
    rpb_d = nc.dram_tensor("rpb", [1, H * L], F32, kind="ExternalInput").ap() if rpb_nz else None
    # scratch for the [1,S] -> [128, NC_T] row->column transpose round-trip
    rm_d = nc.dram_tensor("rm_scratch", [L, 2, 512], F32, kind="Internal").ap()
    out_d = nc.dram_tensor("logits_t", [NC_V, 128, S], F32, kind="ExternalOutput").ap()

    with tile.TileContext(nc) as tc, nc.allow_low_precision(
        reason="bf16 tiles feed the PE; accumulation stays fp32 in PSUM"
    ):
        from contextlib import ExitStack

        with ExitStack() as ctx:
            cp = ctx.enter_context(tc.tile_pool(name="consts", bufs=1))
            xp = ctx.enter_context(tc.tile_pool(name="x", bufs=2))
            xbp = ctx.enter_context(tc.tile_pool(name="xb", bufs=1))
            up = ctx.enter_context(tc.tile_pool(name="u", bufs=1))
            vp = ctx.enter_context(tc.tile_pool(name="v", bufs=1))
            gp = ctx.enter_context(tc.tile_pool(name="g", bufs=1))
            qkp = ctx.enter_context(tc.tile_pool(name="qk", bufs=3))
            tmp = ctx.enter_context(tc.tile_pool(name="tmp", bufs=3))
            rows = ctx.enter_context(tc.tile_pool(name="rows", bufs=4))
            bp_pool = ctx.enter_context(tc.tile_pool(name="bcast", bufs=4))
            atp = ctx.enter_context(tc.tile_pool(name="at", bufs=6))
            wbp = ctx.enter_context(tc.tile_pool(name="wb", bufs=6))
            wvp = ctx.enter_context(tc.tile_pool(name="wvp", bufs=1))
            op_pool = ctx.enter_context(tc.tile_pool(name="out", bufs=4))
            prm = ctx.enter_context(tc.tile_pool(name="prm", bufs=2))
            pmm = ctx.enter_context(tc.tile_pool(name="pmm", bufs=3, space="PSUM"))
            pao = ctx.enter_context(tc.tile_pool(name="pao", bufs=1, space="PSUM"))
            psc = ctx.enter_context(tc.tile_pool(name="psc", bufs=2, space="PSUM"))
            pst = ctx.enter_context(tc.tile_pool(name="pst", bufs=1, space="PSUM"))

            mm = nc.tensor.matmul

            onec = cp.tile([128, 1], DT)
            nc.sync.dma_start(onec[:], onec_d[:])
            tri_t = cp.tile([128, 128], DT)
            nc.sync.dma_start(tri_t[:], tri_d[:])
            if rpb_nz:
                rpb_row = cp.tile([1, H * L], F32)
                nc.sync.dma_start(rpb_row[:], rpb_d[:])
                rpb_t = cp.tile([128, H * L], F32)
                nc.gpsimd.partition_broadcast(rpb_t[:], rpb_row[:])

            x_cur = xp.tile([128, NC_D * S], F32, tag="x")
            for c in range(NC_D):
                nc.sync.dma_start(x_cur[:, ts(c, S)], x0_d[c])
            # bf16 shadow of x: feeds every GEMM, the LN-stats matmuls, and the
            # final logits GEMM
            xb = xbp.tile([128, NC_D * S], DT, tag="xb")
            ps2 = pst.tile([1, 2 * S], F32, tag="st")
            ps_s, ps_q = ps2[:, 0:S], ps2[:, S : 2 * S]
            for c in range(NC_D):
                if c % 2 == 0:
                    nc.scalar.copy(xb[:, ts(c, S)], x_cur[:, ts(c, S)])
                else:
                    nc.vector.tensor_copy(xb[:, ts(c, S)], x_cur[:, ts(c, S)])
                sq = tmp.tile([128, S], DT, tag="sq")
                nc.vector.tensor_mul(sq[:], xb[:, ts(c, S)], xb[:, ts(c, S)])
                mm(ps_s, onec[:], xb[:, ts(c, S)], start=(c == 0), stop=(c == NC_D - 1),
                   skip_group_check=True)
                mm(ps_q, onec[:], sq[:], start=(c == 0), stop=(c == NC_D - 1),
                   skip_group_check=True)

            for l in range(L):
                # ---- per-layer params ----
                ncs_qk = prm.tile([128, 16], F32, tag="ncsqk")
                nc.sync.dma_start(ncs_qk[:], ncs_qk_d[l].rearrange("c p -> p c"))
                ncs_g = prm.tile([128, NC_D], F32, tag="ncsg")
                nc.sync.dma_start(ncs_g[:], ncs_g_d[l].rearrange("c p -> p c"))
                ncs_v = prm.tile([128, 1024], F32, tag="ncsv")
                nc.sync.dma_start(ncs_v[:], ncs_v_d[l])
                if use_bqk:
                    bqk_t = prm.tile([128, 16], F32, tag="bqk")
                    nc.sync.dma_start(bqk_t[:], bqk_d[l].rearrange("c p -> p c"))
                if use_bg:
                    bg_t = prm.tile([128, NC_D], F32, tag="bg")
                    nc.sync.dma_start(bg_t[:], bg_d[l].rearrange("c p -> p c"))
                if use_bo:
                    bo_t = prm.tile([128, NC_D], F32, tag="bo")
                    nc.sync.dma_start(bo_t[:], bo_d[l].rearrange("c p -> p c"))

                # ---- LN factor chain (overlaps the GEMM stream below; nothing
                # on the PE waits for it until first evacuation) ----
                mu = rows.tile([1, S], F32, tag="row")
                nc.vector.tensor_scalar_mul(mu[:], ps_s, 1.0 / D)
                musq = rows.tile([1, S], F32, tag="row")
                nc.vector.tensor_mul(musq[:], mu[:], mu[:])
                var = rows.tile([1, S], F32, tag="row")
                nc.vector.scalar_tensor_tensor(
                    var[:], ps_q, 1.0 / D, musq[:], op0=OP.mult, op1=OP.subtract
                )
                vare = rows.tile([1, S], F32, tag="row")
                nc.vector.tensor_scalar_add(vare[:], var[:], LN_EPS)
                sd = rows.tile([1, S], F32, tag="row")
                nc.scalar.activation(sd[:], vare[:], AF.Sqrt, bias=0.0, scale=1.0)
                rs = rows.tile([1, S], F32, tag="row")
                nc.vector.reciprocal(rs[:], sd[:])
                # row-broadcast [1,S] -> [128,S] on the idle GpSimd engine
                rs_b = bp_pool.tile([128, S], F32, tag="bb")
                nc.gpsimd.partition_broadcast(rs_b[:], rs[:])
                mu_b = bp_pool.tile([128, S], F32, tag="bb")
                nc.gpsimd.partition_broadcast(mu_b[:], mu[:])
                # column orientation [128 tok, NC_T] via DRAM round-trip
                nc.sync.dma_start(rm_d[l, 0:1], rs[:])
                nc.sync.dma_start(rm_d[l, 1:2], mu[:])
                rs_c = prm.tile([128, NC_T], F32, tag="rsc")
                nc.sync.dma_start(
                    rs_c[:], rm_d[l, 0].rearrange("(c p) -> p c", p=128))
                mu_c = prm.tile([128, NC_T], F32, tag="msc")
                nc.sync.dma_start(
                    mu_c[:], rm_d[l, 1].rearrange("(c p) -> p c", p=128))
                if use_bv:
                    vb = []
                    for foB in range(2):
                        bvrow = rows.tile([1, 512], F32, tag="bvr")
                        nc.sync.dma_start(bvrow[:], bv_d[l, foB])
                        vbt = bp_pool.tile([128, 512], F32, tag="vb")
                        nc.gpsimd.partition_broadcast(vbt[:], bvrow[:])
                        vb.append(vbt)

                # ---- V projection, token-major [t, fo] (xb stationary, wv
                # moving); raw evacuation, LN factors applied in place once the
                # factor chain lands ----
                wv_t = wvp.tile([128, 16 * 512], DT, tag="wv")
                for i in range(16):
                    nc.sync.dma_start(wv_t[:, ts(i, 512)], wv_d[l, i])
                v = vp.tile([128, NC_T * 1024], DT, tag="v")
                for foB in range(2):
                    for tc_ in range(NC_T):
                        pv = pmm.tile([128, 512], F32, tag="mm")
                        for fi in range(NC_D):
                            mm(
                                pv[:],
                                xb[:, fi * S + tc_ * 128 : fi * S + tc_ * 128 + 128],
                                wv_t[:, ts(foB * 8 + fi, 512)],
                                start=(fi == 0),
                                stop=(fi == NC_D - 1),
                            )
                        dst = v[:, tc_ * 1024 + foB * 512 : tc_ * 1024 + foB * 512 + 512]
                        nc.scalar.copy(dst, pv[:])
                        nc.vector.scalar_tensor_tensor(
                            dst, ncs_v[:, ts(foB, 512)], mu_c[:, tc_ : tc_ + 1],
                            dst, op0=OP.mult, op1=OP.add,
                        )
                        nc.vector.tensor_scalar_mul(dst, dst, rs_c[:, tc_ : tc_ + 1])
                        if use_bv:
                            nc.vector.tensor_add(dst, dst, vb[foB][:])

                # ---- gate: u = silu(rs*(xb @ gate_w - mu*colsum) + bg); u
                # stays fp32 — it only feeds the DVE gating multiply ----
                u = up.tile([128, NC_D * S], F32, tag="u")
                for f in range(NC_D):
                    wt = wbp.tile([128, 1024], DT, tag="wb")
                    nc.sync.dma_start(wt[:], wg_d[l, f])
                    pu = pmm.tile([128, S], F32, tag="mm")
                    for fi in range(NC_D):
                        mm(pu[:], wt[:, ts(fi, 128)], xb[:, ts(fi, S)],
                           start=(fi == 0), stop=(fi == NC_D - 1))
                    us = u[:, ts(f, S)]
                    nc.vector.scalar_tensor_tensor(
                        us, mu_b[:], ncs_g[:, f : f + 1], pu[:],
                        op0=OP.mult, op1=OP.add,
                    )
                    nc.vector.tensor_mul(us, us, rs_b[:])
                    nc.scalar.activation(
                        us, us, AF.Silu,
                        bias=(bg_t[:, f : f + 1] if use_bg else 0.0), scale=1.0,
                    )

                # ---- attention, one head pair (= one q/k feature chunk) at a
                # time. Scores for the two 64-wide heads issue back-to-back
                # (disjoint PE row groups), AV back-to-back via col tiling into
                # one [128, S] PSUM tile. ----
                g = gp.tile([128, NC_D * S], DT, tag="g")
                for p_ in range(NP):
                    qc = qkp.tile([128, S], DT, tag="qc")
                    kc = qkp.tile([128, S], DT, tag="kc")
                    for (dst, fidx) in ((qc, p_), (kc, 8 + p_)):
                        wt = wbp.tile([128, 1024], DT, tag="wb")
                        nc.sync.dma_start(wt[:], wqk_d[l, fidx])
                        pq = pmm.tile([128, S], F32, tag="mm")
                        for fi in range(NC_D):
                            mm(pq[:], wt[:, ts(fi, 128)], xb[:, ts(fi, S)],
                               start=(fi == 0), stop=(fi == NC_D - 1))
                        nc.vector.scalar_tensor_tensor(
                            dst[:], mu_b[:], ncs_qk[:, fidx : fidx + 1], pq[:],
                            op0=OP.mult, op1=OP.add,
                        )
                        nc.vector.tensor_mul(dst[:], dst[:], rs_b[:])
                        if use_bqk:
                            nc.vector.tensor_scalar_add(
                                dst[:], dst[:], bqk_t[:, fidx : fidx + 1])

                    ao = pao.tile([128, S], F32, tag="ao")
                    for c in range(NC_T):
                        n = S - 128 * c
                        sc_e = psc.tile([128, 512], F32, tag="sc")
                        sc_o = psc.tile([128, 512], F32, tag="sc")
                        mm(sc_e[:, :n], kc[0:64, ts(c, 128)],
                           qc[0:64, c * 128 : S], start=True, stop=True)
                        mm(sc_o[:, :n], kc[64:128, ts(c, 128)],
                           qc[64:128, c * 128 : S], start=True, stop=True)
                        att = {}
                        for (o, hh, sc) in ((0, 2 * p_, sc_e), (64, 2 * p_ + 1, sc_o)):
                            sig_bias = rpb_t[:, l * H + hh : l * H + hh + 1] if rpb_nz else 0.0
                            at = atp.tile([128, S], DT, tag="at")
                            nc.scalar.activation(at[:, 0:n], sc[:, 0:n], AF.Sigmoid,
                                                 bias=sig_bias, scale=DH**-0.5)
                            nc.vector.tensor_mul(at[:, 0:128], at[:, 0:128], tri_t[:])
                            att[o] = at
                        for o in (0, 64):
                            mm(
                                ao[o : o + 64, c * 128 : S],
                                v[:, c * 1024 + p_ * 128 + o : c * 1024 + p_ * 128 + o + 64],
                                att[o][:, 0:n],
                                start=(c == 0),
                                stop=(c == NC_T - 1),
                                tile_position=(0, o),
                            )
                    nc.vector.tensor_mul(g[:, ts(p_, S)], ao[:], u[:, ts(p_, S)])

                # ---- out projection + residual + bf16 shadow refresh; next
                # layer's LN-stats matmuls accumulate here chunk-by-chunk so the
                # PE flows through the layer boundary without a stats stall ----
                x_new = xp.tile([128, NC_D * S], F32, tag="x")
                xb = xbp.tile([128, NC_D * S], DT, tag="xb")
                if l < L - 1:
                    ps2 = pst.tile([1, 2 * S], F32, tag="st")
                    ps_s, ps_q = ps2[:, 0:S], ps2[:, S : 2 * S]
                for f in range(NC_D):
                    wt = wbp.tile([128, 1024], DT, tag="wb")
                    nc.sync.dma_start(wt[:], wo_d[l, f])
                    pd = pmm.tile([128, S], F32, tag="mm")
                    for fi in range(NC_D):
                        mm(pd[:], wt[:, ts(fi, 128)], g[:, ts(fi, S)],
                           start=(fi == 0), stop=(fi == NC_D - 1))
                    if use_bo:
                        t3 = tmp.tile([128, S], F32, tag="tmp")
                        nc.scalar.activation(t3[:], pd[:], AF.Identity,
                                             bias=bo_t[:, f : f + 1], scale=1.0)
                        nc.vector.tensor_add(x_new[:, ts(f, S)], t3[:], x_cur[:, ts(f, S)])
                    else:
                        nc.vector.tensor_add(x_new[:, ts(f, S)], pd[:], x_cur[:, ts(f, S)])
                    nc.scalar.copy(xb[:, ts(f, S)], x_new[:, ts(f, S)])
                    if l < L - 1:
                        sq = tmp.tile([128, S], DT, tag="sq")
                        nc.vector.tensor_mul(sq[:], xb[:, ts(f, S)], xb[:, ts(f, S)])
                        mm(ps_s, onec[:], xb[:, ts(f, S)],
                           start=(f == 0), stop=(f == NC_D - 1), skip_group_check=True)
                        mm(ps_q, onec[:], sq[:], start=(f == 0), stop=(f == NC_D - 1),
                           skip_group_check=True)
                x_cur = x_new

            # ---- logits: [V, T] feature(vocab)-major; xb is the bf16 final x ----
            if use_bp:
                bp_t = cp.tile([128, NC_V], F32)
                nc.sync.dma_start(bp_t[:], bp_d.rearrange("v p -> p v"))
            for vo in range(NC_V):
                wt = wbp.tile([128, 1024], DT, tag="wb")
                nc.sync.dma_start(wt[:], wp_d[vo])
                pl = pmm.tile([128, S], F32, tag="mm")
                for fi in range(NC_D):
                    mm(pl[:], wt[:, ts(fi, 128)], xb[:, ts(fi, S)],
                       start=(fi == 0), stop=(fi == NC_D - 1))
                ot = op_pool.tile([128, S], F32, tag="o")
                if use_bp:
                    nc.scalar.activation(ot[:], pl[:], AF.Identity,
                                         bias=bp_t[:, vo : vo + 1], scale=1.0)
                elif vo % 2 == 0:
                    nc.scalar.copy(ot[:], pl[:])
                else:
                    nc.vector.tensor_copy(ot[:], pl[:])
                nc.sync.dma_start(out_d[vo], ot[:])

    nc.compile()
    return nc


def _get_program(cfg):
    nc = _prog_cache.get(cfg)
    if nc is None:
        nc = _build(cfg)
        _prog_cache[cfg] = nc
    return nc


def _marshal(inputs):
    """Host-side input marshalling into DMA-friendly bf16 layouts."""
    f = np.float32
    input_ids = np.asarray(inputs["input_ids"])
    emb = np.asarray(inputs["embedding"], f)
    pos = np.asarray(inputs["pos_encoding"], f)
    qkv_w = np.asarray(inputs["qkv_w"], f)
    gate_w = np.asarray(inputs["gate_w"], f)
    out_w = np.asarray(inputs["out_w"], f)
    proj_w = np.asarray(inputs["proj_w"], f)
    qkv_b = np.asarray(inputs["qkv_b"], f)
    gate_b = np.asarray(inputs["gate_b"], f)
    out_b = np.asarray(inputs["out_b"], f)
    proj_b = np.asarray(inputs["proj_b"], f)
    ln_g = np.asarray(inputs["ln_g"], f)
    ln_b = np.asarray(inputs["ln_b"], f)
    rpb = np.asarray(inputs["rel_pos_bias"], f)

    x0 = emb[input_ids] + pos[:, :S, :]                       # [B, S, D]
    # feature-major per core: [D, S] -> [NC_D, 128, S]
    x0t = np.ascontiguousarray(x0.transpose(0, 2, 1)).reshape(B, NC_D, 128, S)

    # absorb the LN affine into the LN-consuming projections:
    #   W_eff[f,o] = W[f,o] * ln_g[f];  b_eff[o] = b[o] + ln_b @ W
    qkv_w_eff = qkv_w * ln_g[:, :, None]
    qkv_b_eff = qkv_b + np.einsum("lf,lfo->lo", ln_b, qkv_w)
    gate_w_eff = gate_w * ln_g[:, :, None]
    gate_b_eff = gate_b + np.einsum("lf,lfo->lo", ln_b, gate_w)

    # negated column sums (for the deferred mean subtraction)
    ncs_qk = -qkv_w_eff[:, :, :2048].sum(axis=1)              # [L, 2048]
    ncs_g = -gate_w_eff.sum(axis=1)                           # [L, 1024]
    ncs_v_row = -qkv_w_eff[:, :, 2048:].sum(axis=1)           # [L, 1024]
    ncs_v = np.ascontiguousarray(
        np.broadcast_to(ncs_v_row[:, None, :], (L, 128, 1024)))

    # lhsT tiles [K=fi(128), M=fo(128)] packed 8-fi-wide: [l, fo, 128p, 8c*128j]
    def lhs_tiles(w, nfo):  # w: [L, D, nfo*128]
        r = w.reshape(L, NC_D, 128, nfo, 128).transpose(0, 3, 2, 1, 4)
        return np.ascontiguousarray(r.reshape(L, nfo, 128, NC_D * 128)).astype(BF16_NP)

    wqk = lhs_tiles(qkv_w_eff[:, :, :2048], 16)               # [6,16,128,1024]
    wg = lhs_tiles(gate_w_eff, NC_D)                          # [6,8,128,1024]
    wo = lhs_tiles(out_w, NC_D)                               # [6,8,128,1024]
    # v-section as moving tiles [K=fi(128), N=fo(512)]: [l, foB*8+fi, 128, 512]
    wv = qkv_w_eff[:, :, 2048:].reshape(L, NC_D, 128, 2, 512).transpose(0, 3, 1, 2, 4)
    wv = np.ascontiguousarray(wv.reshape(L, 16, 128, 512)).astype(BF16_NP)
    wp = proj_w.reshape(NC_D, 128, NC_V, 128).transpose(2, 1, 0, 3)
    wp = np.ascontiguousarray(wp.reshape(NC_V, 128, NC_D * 128)).astype(BF16_NP)

    tri = np.triu(np.ones((128, 128), f)).astype(BF16_NP)
    onec = np.ones((128, 1), BF16_NP)

    use_bqk = np.any(qkv_b_eff[:, :2048] != 0.0)
    use_bv = np.any(qkv_b_eff[:, 2048:] != 0.0)
    use_bg = np.any(gate_b_eff != 0.0)
    use_bo = np.any(out_b != 0.0)
    use_bp = np.any(proj_b != 0.0)
    rpb_nz = bool(np.any(rpb != 0.0))

    shared = {
        "wqk": wqk, "wv": wv, "wg": wg, "wo": wo, "wp": wp,
        "ncs_qk": np.ascontiguousarray(ncs_qk.reshape(L, 16, 128)),
        "ncs_g": np.ascontiguousarray(ncs_g.reshape(L, NC_D, 128)),
        "ncs_v": ncs_v,
        "tri": tri, "onec": onec,
    }
    if use_bqk:
        shared["bqk"] = np.ascontiguousarray(qkv_b_eff[:, :2048].reshape(L, 16, 128))
    if use_bv:
        shared["bv"] = np.ascontiguousarray(qkv_b_eff[:, 2048:].reshape(L, 2, 1, 512))
    if use_bg:
        shared["bg"] = np.ascontiguousarray(gate_b_eff.reshape(L, NC_D, 128))
    if use_bo:
        shared["bo"] = np.ascontiguousarray(out_b.reshape(L, NC_D, 128))
    if use_bp:
        shared["bp"] = np.ascontiguousarray(proj_b.reshape(NC_V, 128))
    if rpb_nz:
        shared["rpb"] = np.ascontiguousarray(rpb.reshape(1, L * H))

    cfg = (use_bqk, use_bv, use_bg, use_bo, use_bp, rpb_nz)
    in_maps = []
    for b in range(B):
        m = dict(shared)
        m["x0t"] = np.ascontiguousarray(x0t[b])
        in_maps.append(m)
    return cfg, in_maps


def run(inputs, mm_mode="bf16", trace=False):
    cfg, in_maps = _marshal(inputs)
    nc = _get_program(cfg)
    res = run_bass_kernel_spmd(nc, in_maps, core_ids=list(range(N_CORES)), trace=trace)
    out = np.empty((B, S, V), np.float32)
    for b in range(B):
        lt = res.results[b]["logits_t"].reshape(V, S)
        out[b] = lt.T
    return out, res


def kernel(**inputs) -> np.ndarray:
    out, _ = run(inputs, trace=False)
    return out


# revision 19
# speedup vs baseline: 1.1291x; 1.0038x over previous
"""HSTU-style dense transformer for sequence modeling on 8 Trainium2 NeuronCores.

Sharding: data-parallel over batch (B=8 -> 1 sequence per core). All weights
replicated. Activations are kept feature-major [D=partitions, T=free] on chip;
attention computes transposed scores [kt, qt] so the sigmoid/causal-trim/AV
chain needs no on-device transposes.

All matmul operands are bf16 (weights pre-cast on host): bf16 gets the fast
weight-load path so LDWEIGHTS hides under the previous matmul's streaming, and
weight DMA traffic halves vs f32. The residual stream x and all PSUM
accumulation stay fp32; a bf16 shadow copy of x (xb) feeds every GEMM.

LayerNorm is DEFERRED past the GEMMs: since
    W @ ((x - mu) * rs) = rs * (W @ x) - (rs * mu) * colsum(W),
the qkv/gate projections run directly on raw xb (available the moment the
previous layer's residual lands, so the PE never idles at a layer boundary)
and the per-token factors rs / mu*rs are applied at PSUM evacuation as two
vector ops, using host-precomputed negated column sums. The factor chain
(mean/var/sqrt/reciprocal rows + GpSimd partition-broadcasts + a DRAM-
roundtrip transpose for the token-major V factors) runs concurrently with the
GEMM stream instead of serializing in front of it. The LN affine (ln_g/ln_b)
is absorbed into the projection weights and biases on the host.

Host side only marshals: embedding gather + positional add, weight pre-tiling
into DMA-contiguous bf16 layouts, colsum precompute, and the final
[V,T] -> [S,V] untranspose.
"""

import sys

sys.path.insert(0, "/opt/trn_rl_repo")

import numpy as np

import concourse.bass as bass  # noqa: F401  (keeps bass registered before bacc)
import concourse.tile as tile
from concourse import bacc, mybir
from concourse.bass import ts
from concourse.bass_utils import run_bass_kernel_spmd

B, S, D, H, L, V = 8, 512, 1024, 16, 6, 32000
DH = D // H
LN_EPS = 1e-5
N_CORES = 8
NC_D = D // 128      # 8 feature chunks
NC_T = S // 128      # 4 token chunks
NC_V = V // 128      # 250 vocab chunks
NP = 8               # head pairs

F32 = mybir.dt.float32
BF16 = mybir.dt.bfloat16
BF16_NP = mybir.dt.np(mybir.dt.bfloat16)
AF = mybir.ActivationFunctionType
OP = mybir.AluOpType

_prog_cache = {}


def _build(cfg):
    """Build + compile the SPMD per-core program. cfg is a hashable dict-key."""
    (use_bqk, use_bv, use_bg, use_bo, use_bp, rpb_nz) = cfg
    DT = BF16

    nc = bacc.Bacc("TRN2", target_bir_lowering=False, debug=False)

    x0_d = nc.dram_tensor("x0t", [NC_D, 128, S], DT, kind="ExternalInput").ap()
    wqk_d = nc.dram_tensor("wqk", [L, 16, 128, 1024], DT, kind="ExternalInput").ap()
    wv_d = nc.dram_tensor("wv", [L, 16, 128, 512], DT, kind="ExternalInput").ap()
    wg_d = nc.dram_tensor("wg", [L, NC_D, 128, 1024], DT, kind="ExternalInput").ap()
    wo_d = nc.dram_tensor("wo", [L, NC_D, 128, 1024], DT, kind="ExternalInput").ap()
    wp_d = nc.dram_tensor("wp", [NC_V, 128, 1024], DT, kind="ExternalInput").ap()
    ncs_qk_d = nc.dram_tensor("ncs_qk", [L, 16, 128], F32, kind="ExternalInput").ap()
    ncs_g_d = nc.dram_tensor("ncs_g", [L, NC_D, 128], F32, kind="ExternalInput").ap()
    ncs_v_d = nc.dram_tensor("ncs_v", [L, 128, 1024], F32, kind="ExternalInput").ap()
    tri_d = nc.dram_tensor("tri", [128, 128], DT, kind="ExternalInput").ap()
    onec_d = nc.dram_tensor("onec", [128, 1], DT, kind="ExternalInput").ap()
    bqk_d = nc.dram_tensor("bqk", [L, 16, 128], F32, kind="ExternalInput").ap() if use_bqk else None
    bv_d = nc.dram_tensor("bv", [L, 2, 1, 512], F32, kind="ExternalInput").ap() if use_bv else None
    bg_d = nc.dram_tensor("bg", [L, NC_D, 128], F32, kind="ExternalInput").ap() if use_bg else None
    bo_d = nc.dram_tensor("bo", [L, NC_D, 128], F32, kind="ExternalInput").ap() if use_bo else None
    bp_d = nc.dram_tensor("bp", [NC_V, 128], F32, kind="ExternalInput").ap() if use_bp else None
    rpb_d = nc.dram_tensor("rpb", [1, H * L], F32, kind="ExternalInput").ap() if rpb_nz else None
    # scratch for the [1,S] -> [128, NC_T] row->column transpose round-trip
    rm_d = nc.dram_tensor("rm_scratch", [L, 2, 512], F32, kind="Internal").ap()
    out_d = nc.dram_tensor("logits_t", [NC_V, 128, S], F32, kind="ExternalOutput").ap()

    with tile.TileContext(nc) as tc, nc.allow_low_precision(
        reason="bf16 tiles feed the PE; accumulation stays fp32 in PSUM"
    ):
        from contextlib import ExitStack

        with ExitStack() as ctx:
            cp = ctx.enter_context(tc.tile_pool(name="consts", bufs=1))
            xp = ctx.enter_context(tc.tile_pool(name="x", bufs=2))
            xbp = ctx.enter_context(tc.tile_pool(name="xb", bufs=1))
            up = ctx.enter_context(tc.tile_pool(name="u", bufs=1))
            vp = ctx.enter_context(tc.tile_pool(name="v", bufs=1))
            gp = ctx.enter_context(tc.tile_pool(name="g", bufs=1))
            qkp = ctx.enter_context(tc.tile_pool(name="qk", bufs=3))
            tmp = ctx.enter_context(tc.tile_pool(name="tmp", bufs=3))
            rows = ctx.enter_context(tc.tile_pool(name="rows", bufs=4))
            bp_pool = ctx.enter_context(tc.tile_pool(name="bcast", bufs=4))
            atp = ctx.enter_context(tc.tile_pool(name="at", bufs=6))
            wbp = ctx.enter_context(tc.tile_pool(name="wb", bufs=6))
            wvp = ctx.enter_context(tc.tile_pool(name="wvp", bufs=1))
            op_pool = ctx.enter_context(tc.tile_pool(name="out", bufs=4))
            prm = ctx.enter_context(tc.tile_pool(name="prm", bufs=2))
            pmm = ctx.enter_context(tc.tile_pool(name="pmm", bufs=3, space="PSUM"))
            pao = ctx.enter_context(tc.tile_pool(name="pao", bufs=1, space="PSUM"))
            psc = ctx.enter_context(tc.tile_pool(name="psc", bufs=2, space="PSUM"))
            pst = ctx.enter_context(tc.tile_pool(name="pst", bufs=1, space="PSUM"))

            mm = nc.tensor.matmul

            onec = cp.tile([128, 1], DT)
            nc.sync.dma_start(onec[:], onec_d[:])
            tri_t = cp.tile([128, 128], DT)
            nc.sync.dma_start(tri_t[:], tri_d[:])
            if rpb_nz:
                rpb_row = cp.tile([1, H * L], F32)
                nc.sync.dma_start(rpb_row[:], rpb_d[:])
                rpb_t = cp.tile([128, H * L], F32)
                nc.gpsimd.partition_broadcast(rpb_t[:], rpb_row[:])

            # bf16 shadow of x: feeds every GEMM, the LN-stats matmuls, and the
            # final logits GEMM. x0 arrives as bf16 so xb is a direct DMA and
            # the GEMM stream starts immediately; the f32 residual base is cast
            # off the critical path (first needed at layer 0's residual add).
            xb = xbp.tile([128, NC_D * S], DT, tag="xb")
            for c in range(NC_D):
                nc.sync.dma_start(xb[:, ts(c, S)], x0_d[c])
            x_cur = xp.tile([128, NC_D * S], F32, tag="x")
            ps2 = pst.tile([1, 2 * S], F32, tag="st")
            ps_s, ps_q = ps2[:, 0:S], ps2[:, S : 2 * S]
            for c in range(NC_D):
                if c % 2 == 0:
                    nc.scalar.copy(x_cur[:, ts(c, S)], xb[:, ts(c, S)])
                else:
                    nc.vector.tensor_copy(x_cur[:, ts(c, S)], xb[:, ts(c, S)])
                sq = tmp.tile([128, S], DT, tag="sq")
                nc.vector.tensor_mul(sq[:], xb[:, ts(c, S)], xb[:, ts(c, S)])
                mm(ps_s, onec[:], xb[:, ts(c, S)], start=(c == 0), stop=(c == NC_D - 1),
                   skip_group_check=True)
                mm(ps_q, onec[:], sq[:], start=(c == 0), stop=(c == NC_D - 1),
                   skip_group_check=True)

            for l in range(L):
                # ---- per-layer params ----
                ncs_qk = prm.tile([128, 16], F32, tag="ncsqk")
                nc.sync.dma_start(ncs_qk[:], ncs_qk_d[l].rearrange("c p -> p c"))
                ncs_g = prm.tile([128, NC_D], F32, tag="ncsg")
                nc.sync.dma_start(ncs_g[:], ncs_g_d[l].rearrange("c p -> p c"))
                ncs_v = prm.tile([128, 1024], F32, tag="ncsv")
                nc.sync.dma_start(ncs_v[:], ncs_v_d[l])
                if use_bqk:
                    bqk_t = prm.tile([128, 16], F32, tag="bqk")
                    nc.sync.dma_start(bqk_t[:], bqk_d[l].rearrange("c p -> p c"))
                if use_bg:
                    bg_t = prm.tile([128, NC_D], F32, tag="bg")
                    nc.sync.dma_start(bg_t[:], bg_d[l].rearrange("c p -> p c"))
                if use_bo:
                    bo_t = prm.tile([128, NC_D], F32, tag="bo")
                    nc.sync.dma_start(bo_t[:], bo_d[l].rearrange("c p -> p c"))

                # ---- LN factor chain (overlaps the GEMM stream below; nothing
                # on the PE waits for it until first evacuation) ----
                mu = rows.tile([1, S], F32, tag="row")
                nc.vector.tensor_scalar_mul(mu[:], ps_s, 1.0 / D)
                musq = rows.tile([1, S], F32, tag="row")
                nc.vector.tensor_mul(musq[:], mu[:], mu[:])
                var = rows.tile([1, S], F32, tag="row")
                nc.vector.scalar_tensor_tensor(
                    var[:], ps_q, 1.0 / D, musq[:], op0=OP.mult, op1=OP.subtract
                )
                vare = rows.tile([1, S], F32, tag="row")
                nc.vector.tensor_scalar_add(vare[:], var[:], LN_EPS)
                sd = rows.tile([1, S], F32, tag="row")
                nc.scalar.activation(sd[:], vare[:], AF.Sqrt, bias=0.0, scale=1.0)
                rs = rows.tile([1, S], F32, tag="row")
                nc.vector.reciprocal(rs[:], sd[:])
                # row-broadcast [1,S] -> [128,S] on the idle GpSimd engine
                rs_b = bp_pool.tile([128, S], F32, tag="bb")
                nc.gpsimd.partition_broadcast(rs_b[:], rs[:])
                mu_b = bp_pool.tile([128, S], F32, tag="bb")
                nc.gpsimd.partition_broadcast(mu_b[:], mu[:])
                # column orientation [128 tok, NC_T] via DRAM round-trip
                nc.sync.dma_start(rm_d[l, 0:1], rs[:])
                nc.sync.dma_start(rm_d[l, 1:2], mu[:])
                rs_c = prm.tile([128, NC_T], F32, tag="rsc")
                nc.sync.dma_start(
                    rs_c[:], rm_d[l, 0].rearrange("(c p) -> p c", p=128))
                mu_c = prm.tile([128, NC_T], F32, tag="msc")
                nc.sync.dma_start(
                    mu_c[:], rm_d[l, 1].rearrange("(c p) -> p c", p=128))
                if use_bv:
                    vb = []
                    for foB in range(2):
                        bvrow = rows.tile([1, 512], F32, tag="bvr")
                        nc.sync.dma_start(bvrow[:], bv_d[l, foB])
                        vbt = bp_pool.tile([128, 512], F32, tag="vb")
                        nc.gpsimd.partition_broadcast(vbt[:], bvrow[:])
                        vb.append(vbt)

                # ---- V projection, token-major [t, fo] (xb stationary, wv
                # moving); raw evacuation, LN factors applied in place once the
                # factor chain lands ----
                wv_t = wvp.tile([128, 16 * 512], DT, tag="wv")
                for i in range(16):
                    nc.sync.dma_start(wv_t[:, ts(i, 512)], wv_d[l, i])
                v = vp.tile([128, NC_T * 1024], DT, tag="v")
                for foB in range(2):
                    for tc_ in range(NC_T):
                        pv = pmm.tile([128, 512], F32, tag="mm")
                        for fi in range(NC_D):
                            mm(
                                pv[:],
                                xb[:, fi * S + tc_ * 128 : fi * S + tc_ * 128 + 128],
                                wv_t[:, ts(foB * 8 + fi, 512)],
                                start=(fi == 0),
                                stop=(fi == NC_D - 1),
                            )
                        dst = v[:, tc_ * 1024 + foB * 512 : tc_ * 1024 + foB * 512 + 512]
                        nc.scalar.copy(dst, pv[:])
                        nc.vector.scalar_tensor_tensor(
                            dst, ncs_v[:, ts(foB, 512)], mu_c[:, tc_ : tc_ + 1],
                            dst, op0=OP.mult, op1=OP.add,
                        )
                        nc.vector.tensor_scalar_mul(dst, dst, rs_c[:, tc_ : tc_ + 1])
                        if use_bv:
                            nc.vector.tensor_add(dst, dst, vb[foB][:])

                # ---- gate: u = silu(rs*(xb @ gate_w - mu*colsum) + bg); u
                # stays fp32 — it only feeds the DVE gating multiply ----
                u = up.tile([128, NC_D * S], F32, tag="u")
                for f in range(NC_D):
                    wt = wbp.tile([128, 1024], DT, tag="wb")
                    nc.sync.dma_start(wt[:], wg_d[l, f])
                    pu = pmm.tile([128, S], F32, tag="mm")
                    for fi in range(NC_D):
                        mm(pu[:], wt[:, ts(fi, 128)], xb[:, ts(fi, S)],
                           start=(fi == 0), stop=(fi == NC_D - 1))
                    us = u[:, ts(f, S)]
                    nc.vector.scalar_tensor_tensor(
                        us, mu_b[:], ncs_g[:, f : f + 1], pu[:],
                        op0=OP.mult, op1=OP.add,
                    )
                    nc.vector.tensor_mul(us, us, rs_b[:])
                    nc.scalar.activation(
                        us, us, AF.Silu,
                        bias=(bg_t[:, f : f + 1] if use_bg else 0.0), scale=1.0,
                    )

                # ---- attention, one head pair (= one q/k feature chunk) at a
                # time. Scores for the two 64-wide heads issue back-to-back
                # (disjoint PE row groups), AV back-to-back via col tiling into
                # one [128, S] PSUM tile. ----
                g = gp.tile([128, NC_D * S], DT, tag="g")
                for p_ in range(NP):
                    qc = qkp.tile([128, S], DT, tag="qc")
                    kc = qkp.tile([128, S], DT, tag="kc")
                    for (dst, fidx) in ((qc, p_), (kc, 8 + p_)):
                        wt = wbp.tile([128, 1024], DT, tag="wb")
                        nc.sync.dma_start(wt[:], wqk_d[l, fidx])
                        pq = pmm.tile([128, S], F32, tag="mm")
                        for fi in range(NC_D):
                            mm(pq[:], wt[:, ts(fi, 128)], xb[:, ts(fi, S)],
                               start=(fi == 0), stop=(fi == NC_D - 1))
                        nc.vector.scalar_tensor_tensor(
                            dst[:], mu_b[:], ncs_qk[:, fidx : fidx + 1], pq[:],
                            op0=OP.mult, op1=OP.add,
                        )
                        nc.vector.tensor_mul(dst[:], dst[:], rs_b[:])
                        if use_bqk:
                            nc.vector.tensor_scalar_add(
                                dst[:], dst[:], bqk_t[:, fidx : fidx + 1])

                    ao = pao.tile([128, S], F32, tag="ao")
                    for c in range(NC_T):
                        n = S - 128 * c
                        sc_e = psc.tile([128, 512], F32, tag="sc")
                        sc_o = psc.tile([128, 512], F32, tag="sc")
                        mm(sc_e[:, :n], kc[0:64, ts(c, 128)],
                           qc[0:64, c * 128 : S], start=True, stop=True)
                        mm(sc_o[:, :n], kc[64:128, ts(c, 128)],
                           qc[64:128, c * 128 : S], start=True, stop=True)
                        att = {}
                        for (o, hh, sc) in ((0, 2 * p_, sc_e), (64, 2 * p_ + 1, sc_o)):
                            sig_bias = rpb_t[:, l * H + hh : l * H + hh + 1] if rpb_nz else 0.0
                            at = atp.tile([128, S], DT, tag="at")
                            nc.scalar.activation(at[:, 0:n], sc[:, 0:n], AF.Sigmoid,
                                                 bias=sig_bias, scale=DH**-0.5)
                            nc.vector.tensor_mul(at[:, 0:128], at[:, 0:128], tri_t[:])
                            att[o] = at
                        for o in (0, 64):
                            mm(
                                ao[o : o + 64, c * 128 : S],
                                v[:, c * 1024 + p_ * 128 + o : c * 1024 + p_ * 128 + o + 64],
                                att[o][:, 0:n],
                                start=(c == 0),
                                stop=(c == NC_T - 1),
                                tile_position=(0, o),
                            )
                    nc.vector.tensor_mul(g[:, ts(p_, S)], ao[:], u[:, ts(p_, S)])

                # ---- out projection + residual + bf16 shadow refresh; next
                # layer's LN-stats matmuls accumulate here chunk-by-chunk so the
                # PE flows through the layer boundary without a stats stall ----
                x_new = xp.tile([128, NC_D * S], F32, tag="x")
                xb = xbp.tile([128, NC_D * S], DT, tag="xb")
                if l < L - 1:
                    ps2 = pst.tile([1, 2 * S], F32, tag="st")
                    ps_s, ps_q = ps2[:, 0:S], ps2[:, S : 2 * S]
                for f in range(NC_D):
                    wt = wbp.tile([128, 1024], DT, tag="wb")
                    nc.sync.dma_start(wt[:], wo_d[l, f])
                    pd = pmm.tile([128, S], F32, tag="mm")
                    for fi in range(NC_D):
                        mm(pd[:], wt[:, ts(fi, 128)], g[:, ts(fi, S)],
                           start=(fi == 0), stop=(fi == NC_D - 1))
                    if use_bo:
                        t3 = tmp.tile([128, S], F32, tag="tmp")
                        nc.scalar.activation(t3[:], pd[:], AF.Identity,
                                             bias=bo_t[:, f : f + 1], scale=1.0)
                        nc.vector.tensor_add(x_new[:, ts(f, S)], t3[:], x_cur[:, ts(f, S)])
                    else:
                        nc.vector.tensor_add(x_new[:, ts(f, S)], pd[:], x_cur[:, ts(f, S)])
                    nc.scalar.copy(xb[:, ts(f, S)], x_new[:, ts(f, S)])
                    if l < L - 1:
                        sq = tmp.tile([128, S], DT, tag="sq")
                        nc.vector.tensor_mul(sq[:], xb[:, ts(f, S)], xb[:, ts(f, S)])
                        mm(ps_s, onec[:], xb[:, ts(f, S)],
                           start=(f == 0), stop=(f == NC_D - 1), skip_group_check=True)
                        mm(ps_q, onec[:], sq[:], start=(f == 0), stop=(f == NC_D - 1),
                           skip_group_check=True)
                x_cur = x_new

            # ---- logits: [V, T] feature(vocab)-major; xb is the bf16 final x ----
            if use_bp:
                bp_t = cp.tile([128, NC_V], F32)
                nc.sync.dma_start(bp_t[:], bp_d.rearrange("v p -> p v"))
            for vo in range(NC_V):
                wt = wbp.tile([128, 1024], DT, tag="wb")
                nc.sync.dma_start(wt[:], wp_d[vo])
                pl = pmm.tile([128, S], F32, tag="mm")
                for fi in range(NC_D):
                    mm(pl[:], wt[:, ts(fi, 128)], xb[:, ts(fi, S)],
                       start=(fi == 0), stop=(fi == NC_D - 1))
                ot = op_pool.tile([128, S], F32, tag="o")
                if use_bp:
                    nc.scalar.activation(ot[:], pl[:], AF.Identity,
                                         bias=bp_t[:, vo : vo + 1], scale=1.0)
                elif vo % 2 == 0:
                    nc.scalar.copy(ot[:], pl[:])
                else:
                    nc.vector.tensor_copy(ot[:], pl[:])
                nc.sync.dma_start(out_d[vo], ot[:])

    nc.compile()
    return nc


def _get_program(cfg):
    nc = _prog_cache.get(cfg)
    if nc is None:
        nc = _build(cfg)
        _prog_cache[cfg] = nc
    return nc


def _marshal(inputs):
    """Host-side input marshalling into DMA-friendly bf16 layouts."""
    f = np.float32
    input_ids = np.asarray(inputs["input_ids"])
    emb = np.asarray(inputs["embedding"], f)
    pos = np.asarray(inputs["pos_encoding"], f)
    qkv_w = np.asarray(inputs["qkv_w"], f)
    gate_w = np.asarray(inputs["gate_w"], f)
    out_w = np.asarray(inputs["out_w"], f)
    proj_w = np.asarray(inputs["proj_w"], f)
    qkv_b = np.asarray(inputs["qkv_b"], f)
    gate_b = np.asarray(inputs["gate_b"], f)
    out_b = np.asarray(inputs["out_b"], f)
    proj_b = np.asarray(inputs["proj_b"], f)
    ln_g = np.asarray(inputs["ln_g"], f)
    ln_b = np.asarray(inputs["ln_b"], f)
    rpb = np.asarray(inputs["rel_pos_bias"], f)

    x0 = emb[input_ids] + pos[:, :S, :]                       # [B, S, D]
    # feature-major per core: [D, S] -> [NC_D, 128, S], shipped as bf16
    x0t = np.ascontiguousarray(x0.transpose(0, 2, 1)).reshape(B, NC_D, 128, S).astype(BF16_NP)

    # absorb the LN affine into the LN-consuming projections:
    #   W_eff[f,o] = W[f,o] * ln_g[f];  b_eff[o] = b[o] + ln_b @ W
    qkv_w_eff = qkv_w * ln_g[:, :, None]
    qkv_b_eff = qkv_b + np.einsum("lf,lfo->lo", ln_b, qkv_w)
    gate_w_eff = gate_w * ln_g[:, :, None]
    gate_b_eff = gate_b + np.einsum("lf,lfo->lo", ln_b, gate_w)

    # negated column sums (for the deferred mean subtraction)
    ncs_qk = -qkv_w_eff[:, :, :2048].sum(axis=1)              # [L, 2048]
    ncs_g = -gate_w_eff.sum(axis=1)                           # [L, 1024]
    ncs_v_row = -qkv_w_eff[:, :, 2048:].sum(axis=1)           # [L, 1024]
    ncs_v = np.ascontiguousarray(
        np.broadcast_to(ncs_v_row[:, None, :], (L, 128, 1024)))

    # lhsT tiles [K=fi(128), M=fo(128)] packed 8-fi-wide: [l, fo, 128p, 8c*128j]
    def lhs_tiles(w, nfo):  # w: [L, D, nfo*128]
        r = w.reshape(L, NC_D, 128, nfo, 128).transpose(0, 3, 2, 1, 4)
        return np.ascontiguousarray(r.reshape(L, nfo, 128, NC_D * 128)).astype(BF16_NP)

    wqk = lhs_tiles(qkv_w_eff[:, :, :2048], 16)               # [6,16,128,1024]
    wg = lhs_tiles(gate_w_eff, NC_D)                          # [6,8,128,1024]
    wo = lhs_tiles(out_w, NC_D)                               # [6,8,128,1024]
    # v-section as moving tiles [K=fi(128), N=fo(512)]: [l, foB*8+fi, 128, 512]
    wv = qkv_w_eff[:, :, 2048:].reshape(L, NC_D, 128, 2, 512).transpose(0, 3, 1, 2, 4)
    wv = np.ascontiguousarray(wv.reshape(L, 16, 128, 512)).astype(BF16_NP)
    wp = proj_w.reshape(NC_D, 128, NC_V, 128).transpose(2, 1, 0, 3)
    wp = np.ascontiguousarray(wp.reshape(NC_V, 128, NC_D * 128)).astype(BF16_NP)

    tri = np.triu(np.ones((128, 128), f)).astype(BF16_NP)
    onec = np.ones((128, 1), BF16_NP)

    use_bqk = np.any(qkv_b_eff[:, :2048] != 0.0)
    use_bv = np.any(qkv_b_eff[:, 2048:] != 0.0)
    use_bg = np.any(gate_b_eff != 0.0)
    use_bo = np.any(out_b != 0.0)
    use_bp = np.any(proj_b != 0.0)
    rpb_nz = bool(np.any(rpb != 0.0))

    shared = {
        "wqk": wqk, "wv": wv, "wg": wg, "wo": wo, "wp": wp,
        "ncs_qk": np.ascontiguousarray(ncs_qk.reshape(L, 16, 128)),
        "ncs_g": np.ascontiguousarray(ncs_g.reshape(L, NC_D, 128)),
        "ncs_v": ncs_v,
        "tri": tri, "onec": onec,
    }
    if use_bqk:
        shared["bqk"] = np.ascontiguousarray(qkv_b_eff[:, :2048].reshape(L, 16, 128))
    if use_bv:
        shared["bv"] = np.ascontiguousarray(qkv_b_eff[:, 2048:].reshape(L, 2, 1, 512))
    if use_bg:
        shared["bg"] = np.ascontiguousarray(gate_b_eff.reshape(L, NC_D, 128))
    if use_bo:
        shared["bo"] = np.ascontiguousarray(out_b.reshape(L, NC_D, 128))
    if use_bp:
        shared["bp"] = np.ascontiguousarray(proj_b.reshape(NC_V, 128))
    if rpb_nz:
        shared["rpb"] = np.ascontiguousarray(rpb.reshape(1, L * H))

    cfg = (use_bqk, use_bv, use_bg, use_bo, use_bp, rpb_nz)
    in_maps = []
    for b in range(B):
        m = dict(shared)
        m["x0t"] = np.ascontiguousarray(x0t[b])
        in_maps.append(m)
    return cfg, in_maps


def run(inputs, mm_mode="bf16", trace=False):
    cfg, in_maps = _marshal(inputs)
    nc = _get_program(cfg)
    res = run_bass_kernel_spmd(nc, in_maps, core_ids=list(range(N_CORES)), trace=trace)
    out = np.empty((B, S, V), np.float32)
    for b in range(B):
        lt = res.results[b]["logits_t"].reshape(V, S)
        out[b] = lt.T
    return out, res


def kernel(**inputs) -> np.ndarray:
    out, _ = run(inputs, trace=False)
    return out
